# revision 1
# baseline (speedup 1.0000x reference)
"""AttentionPooling Trainium2 kernel.

Problem (per full input):
    hidden [B=8, S=8192, DM=1024] f32, mask [B, S] bool, query [K=8, DM] f32
    logits = einsum('kd,bsd->bks', query, hidden); masked (-1e4) softmax over S
    out    = einsum('bks,bsd->bkd', attn, hidden)              -> [B, K, DM] f32

Sharding: data-parallel over batch B; core i handles batch i. No collectives.

Precision strategy: bf16 hi/lo split (h = h_hi + h_lo with h_hi = bf16(h),
h_lo = bf16(h - h_hi), same for q and attn weights p). All matmuls run in
bf16 (1 cyc/row on PE vs 4 for fp32) with fp32 PSUM accumulation; keeping all
four cross terms gives ~8e-5 relative error end to end (validated on HW).

Host staging ships BOTH layouts (natural [S,D] for the weighted sum and
transposed [D,S] for the logits matmul), each as a packed hi/lo pair, so the
kernel needs no on-chip transposes of the big tensor and no PSUM round trips.
Per-core DMA is 64 MB in 4 MB transfers (two s-tiles per DMA).

PE packing: the small operand (q for mm1, attn p for mm2) is packed twice into
the stationary at column groups {0:8 hi, 32:40 lo} and {64:72 hi, 96:104 lo};
the hi pass issues at tile_position (0,0) and the lo pass at (0,64), so the
two passes can run on disjoint PE column groups and their PSUM accumulation
groups are partition-disjoint in the same bank. The four row-bands are summed
with cheap [8,*] ops.
"""

import sys

import numpy as np

sys.path.insert(0, "/opt/trn_rl_repo")

import ml_dtypes

import concourse.tile as tile
from concourse import bacc, mybir

FP = mybir.dt.float32
BF = mybir.dt.bfloat16
BF_NP = ml_dtypes.bfloat16

# Problem config (hardcoded; harness calls kernel() with exactly these shapes)
B, S, DM, K = 8, 8192, 1024, 8
N_CORES = 8
NEG_BIG = -30000.0  # additive mask penalty (<= -20000 guarantees exp -> 0)
M_INIT = -20000.0   # initial running max; > mask penalty so exp never overflows
KW = 104            # packed stationary width: hi 0:8 / lo 32:40 / hi 64:72 / lo 96:104
GRP = (0, 64)       # column-group base for the hi / lo pass


def o_acc_band(o_ps, g, k=K):
    return o_ps[g : g + k, :]


def build_program(s=S, dm=DM, k=K, st=512, pair=2):
    """Build the per-core Bass program. Returns the compiled Bacc module."""
    assert s % (st * pair) == 0 and st % 128 == 0 and dm % 512 == 0
    n_tiles = s // st
    n_pairs = n_tiles // pair
    sub = st // 128            # 128-row subchunks per s-tile
    ncd = dm // 128            # d-chunks for mm1
    ndh = dm // 512            # 512-wide d halves for mm2
    kw = KW

    nc = bacc.Bacc(
        "TRN2",
        target_bir_lowering=False,
        debug=False,
        num_devices=N_CORES,
    )

    hT_pack = nc.dram_tensor(
        "hT_pack", [n_tiles, 2 * ncd, 128, st], BF, kind="ExternalInput"
    ).ap()
    h_pack = nc.dram_tensor(
        "h_pack", [n_tiles, 2 * sub, 128, dm], BF, kind="ExternalInput"
    ).ap()
    qT_pack = nc.dram_tensor("qT_pack", [dm, kw], BF, kind="ExternalInput").ap()
    addend = nc.dram_tensor("addend", [1, s], BF, kind="ExternalInput").ap()
    ident = nc.dram_tensor("ident", [kw, kw], BF, kind="ExternalInput").ap()
    negM = nc.dram_tensor("negM", [k, 1], FP, kind="ExternalInput").ap()
    out = nc.dram_tensor("out", [k, dm], FP, kind="ExternalOutput").ap()

    with tile.TileContext(nc) as tc:
        with (
            tc.tile_pool(name="const", bufs=1) as const_pool,
            tc.tile_pool(name="state", bufs=1) as state_pool,
            tc.tile_pool(name="hT", bufs=2) as hT_pool,
            tc.tile_pool(name="hnat", bufs=2) as hnat_pool,
            tc.tile_pool(name="psL", bufs=2, space="PSUM") as psL_pool,
            tc.tile_pool(name="psO", bufs=2, space="PSUM") as psO_pool,
            tc.tile_pool(name="psP", bufs=2, space="PSUM") as psP_pool,
            tc.tile_pool(name="ptile", bufs=2) as p_pool,
            tc.tile_pool(name="small", bufs=4) as small_pool,
        ):
            # ---- constants / persistent state ----
            qT_sb = const_pool.tile([128, ncd * kw], BF, tag="qT")
            nc.sync.dma_start(
                out=qT_sb[:].rearrange("p (j k) -> p j k", j=ncd),
                in_=qT_pack.rearrange("(j p) k -> p j k", p=128),
            )
            addend_sb = const_pool.tile([1, s], BF, tag="addend")
            nc.sync.dma_start(out=addend_sb[:], in_=addend[:])
            ident_sb = const_pool.tile([kw, kw], BF, tag="ident")
            nc.sync.dma_start(out=ident_sb[:], in_=ident[:])
            ones_sb = const_pool.tile([1, kw], BF, tag="ones")
            nc.vector.memset(ones_sb[:], 1.0)

            negM_sb = const_pool.tile([k, 1], FP, tag="negM")
            nc.sync.dma_start(out=negM_sb[:], in_=negM)
            denom = state_pool.tile([k, 1], FP, tag="denom")
            nc.vector.memset(denom[:], 0.0)
            # mm2 accumulates into one persistent PSUM tile across all tiles
            o_ps = psO_pool.tile([kw, dm], FP, tag="psO")

            for tp in range(n_pairs):
                # ---- one DMA per pair of s-tiles (4 MB each) ----
                hT = hT_pool.tile([128, pair * 2 * ncd * st], BF, tag="hT")
                nc.sync.dma_start(
                    out=hT[:].rearrange("p (g s) -> p g s", g=pair * 2 * ncd),
                    in_=hT_pack[tp * pair : (tp + 1) * pair].rearrange(
                        "t vj p s -> p (t vj) s"
                    ),
                )
                h_nat = hnat_pool.tile([128, pair * 2 * sub * dm], BF, tag="h_nat")
                nc.sync.dma_start(
                    out=h_nat[:].rearrange("p (g d) -> p g d", g=pair * 2 * sub),
                    in_=h_pack[tp * pair : (tp + 1) * pair].rearrange(
                        "t vc p d -> p (t vc) d"
                    ),
                )

                for ti in range(pair):
                    t = tp * pair + ti

                    def hT_sl(j, v):
                        base = ((ti * 2 + v) * ncd + j) * st
                        return hT[:, base : base + st]

                    def hnat_sl(c, v, dh):
                        base = ((ti * 2 + v) * sub + c) * dm + dh * 512
                        return h_nat[:, base : base + 512]

                    # ---- mm1: two column-group passes (hi at 0, lo at 64) ----
                    L = psL_pool.tile([kw, st], FP, tag="psL")
                    for v in range(2):
                        g = GRP[v]
                        for j in range(ncd):
                            nc.tensor.matmul(
                                L[g : g + 40, :],
                                qT_sb[:, j * kw + g : j * kw + g + 40],
                                hT_sl(j, v),
                                start=(j == 0),
                                stop=False,
                                tile_position=(0, g),
                            )
                        if v == 0:
                            nc.tensor.matmul(
                                L[0:40, :],
                                ones_sb[:, 0:40],
                                addend_sb[:, t * st : (t + 1) * st],
                                start=False,
                                stop=True,
                                tile_position=(0, 0),
                            )
                        else:
                            nc.tensor.matmul(
                                L[64:104, :],
                                ones_sb[:, 0:40],
                                addend_sb[:, t * st : (t + 1) * st],
                                start=False,
                                stop=True,
                                tile_position=(0, 64),
                            )

                    # ---- Lsum over the four row-bands ----
                    # (base-shifting PSUM->SB copies, then equal-base adds;
                    # note both group sums include the mask addend, so Lsum
                    # carries 2x addend — still <= -40000 on masked cols)
                    Lsum = p_pool.tile([k, st], FP, tag="Lsum")
                    La = p_pool.tile([k, st], FP, tag="La")
                    Lb = p_pool.tile([k, st], FP, tag="Lb")
                    Lc = p_pool.tile([k, st], FP, tag="Lc")
                    nc.scalar.copy(Lsum[:], L[0:k, :])
                    nc.scalar.copy(La[:], L[32 : 32 + k, :])
                    nc.scalar.copy(Lb[:], L[64 : 64 + k, :])
                    nc.scalar.copy(Lc[:], L[96 : 96 + k, :])
                    nc.vector.tensor_add(Lsum[:], Lsum[:], La[:])
                    nc.vector.tensor_add(Lb[:], Lb[:], Lc[:])
                    nc.vector.tensor_add(Lsum[:], Lsum[:], Lb[:])

                    # ---- p = exp(Lsum - M); M is a host-computed per-row
                    # upper bound (sampled logits + margin), so no running
                    # max / rescale chain is needed ----
                    p_sb = p_pool.tile([k, st], FP, tag="p_sb")
                    tsum = small_pool.tile([k, 1], FP, tag="tsum")
                    nc.scalar.activation(
                        p_sb[:],
                        Lsum[:],
                        mybir.ActivationFunctionType.Exp,
                        bias=negM_sb[:],
                        accum_out=tsum[:],
                    )
                    nc.vector.tensor_add(denom[:], denom[:], tsum[:])

                    # ---- split p into [phi|plo] rows, transpose to pT ----
                    p2 = p_pool.tile([40, st], BF, tag="p2")
                    nc.vector.memset(p2[:], 0.0)
                    nc.vector.tensor_copy(p2[0:k, :], p_sb[:])       # phi
                    nc.vector.tensor_sub(
                        p2[32 : 32 + k, :], p_sb[:], p2[0:k, :]
                    )                                                 # plo
                    pT = p_pool.tile([128, sub * 40], BF, tag="pT")
                    for c in range(sub):
                        tpp = psP_pool.tile([128, 40], BF, tag="psP")
                        nc.tensor.transpose(
                            tpp[:],
                            p2[:, c * 128 : (c + 1) * 128],
                            ident_sb[0:40, 0:40],
                        )
                        nc.scalar.copy(pT[:, c * 40 : (c + 1) * 40], tpp[:])

                    # ---- mm2: accumulate into the persistent PSUM group ----
                    # (band B at partitions 64:104 shares the zero region with
                    # the still-open band-A group; the bands are partition-
                    # disjoint, so skip the region bookkeeping for band B)
                    for dh in range(ndh):
                        for v in range(2):
                            g = GRP[v]
                            for c in range(sub):
                                nc.tensor.matmul(
                                    o_ps[g : g + 40, dh * 512 : (dh + 1) * 512],
                                    pT[:, c * 40 : (c + 1) * 40],
                                    hnat_sl(c, v, dh),
                                    start=(t == 0 and c == 0),
                                    stop=(t == n_tiles - 1 and c == sub - 1),
                                    tile_position=(0, g),
                                    skip_group_check=(v == 1),
                                )

            # ---- finalize: out = sum of the four PSUM bands / denom ----
            osum = state_pool.tile([k, dm], FP, tag="osum")
            ot = state_pool.tile([k, dm], FP, tag="ot")
            ot2 = state_pool.tile([k, dm], FP, tag="ot2")
            ot3 = state_pool.tile([k, dm], FP, tag="ot3")
            nc.scalar.copy(osum[:], o_acc_band(o_ps, 0))
            nc.scalar.copy(ot[:], o_acc_band(o_ps, 32))
            nc.scalar.copy(ot2[:], o_acc_band(o_ps, 64))
            nc.scalar.copy(ot3[:], o_acc_band(o_ps, 96))
            nc.vector.tensor_add(osum[:], osum[:], ot[:])
            nc.vector.tensor_add(ot2[:], ot2[:], ot3[:])
            nc.vector.tensor_add(osum[:], osum[:], ot2[:])
            rden = small_pool.tile([k, 1], FP, tag="rden")
            nc.vector.reciprocal(rden[:], denom[:])
            out_sb = state_pool.tile([k, dm], FP, tag="out_sb")
            nc.scalar.activation(
                out_sb[:],
                osum[:],
                mybir.ActivationFunctionType.Copy,
                scale=rden[:],
            )
            nc.sync.dma_start(out=out, in_=out_sb[:])

    nc.compile()
    return nc


_CACHED = {}


def _get_program(key, **kw):
    if key not in _CACHED:
        _CACHED[key] = build_program(**kw)
    return _CACHED[key]


def _split_bf16(x):
    hi = x.astype(BF_NP)
    lo = (x - hi.astype(np.float32)).astype(BF_NP)
    return hi, lo


def make_in_maps(hidden, mask, query):
    """Host-side staging: shard over batch; ship bf16 hi/lo in both layouts."""
    hidden = np.ascontiguousarray(hidden, dtype=np.float32)
    mask = np.asarray(mask)
    query = np.asarray(query, dtype=np.float32)
    b, s, dm = hidden.shape
    k = query.shape[0]

    q_hi, q_lo = _split_bf16(query)                    # [K, DM]
    qT_pack = np.zeros((dm, KW), dtype=BF_NP)
    for g in GRP:
        qT_pack[:, g : g + k] = q_hi.T
        qT_pack[:, g + 32 : g + 32 + k] = q_lo.T
    addend = np.where(mask, 0.0, NEG_BIG).astype(BF_NP)  # [B, S]
    ident = np.eye(KW, dtype=BF_NP)

    # Per-row exp-shift bound M from a 512-row logit sample (+30 margin).
    # true_max - M stays within about +/-35 on N(0,1)-scale data, far inside
    # the fp32 exp range, so no running max is needed on-chip.
    rngM = np.random.default_rng(12345)
    idxM = rngM.choice(s, min(512, s), replace=False)
    negM_all = []
    for i in range(b):
        ls = query @ hidden[i][idxM].T                 # [K, 512]
        ls = np.where(mask[i][idxM][None, :], ls, 2.0 * NEG_BIG)
        M = np.maximum(ls.max(axis=1) + 30.0, 60.0)
        negM_all.append((-M).astype(np.float32).reshape(k, 1))

    st = 512
    n_tiles = s // st
    sub = st // 128
    ncd = dm // 128
    in_maps = []
    for i in range(b):
        h_hi, h_lo = _split_bf16(hidden[i])            # [S, DM] each
        # h_pack [T, 2*sub, 128, DM]: vc = v*sub + c, rows t*st + c*128 + p
        h_pack = np.concatenate(
            [h_hi.reshape(n_tiles, sub, 128, dm),
             h_lo.reshape(n_tiles, sub, 128, dm)],
            axis=1,
        )
        # hT_pack [T, 2*ncd, 128, st]: vj = v*ncd + j, d = j*128 + p
        hT = np.concatenate(
            [np.ascontiguousarray(h_hi.T).reshape(ncd, 128, n_tiles, st),
             np.ascontiguousarray(h_lo.T).reshape(ncd, 128, n_tiles, st)],
            axis=0,
        )
        hT_pack = hT.transpose(2, 0, 1, 3)             # [T, 2*ncd, 128, st]
        in_maps.append(
            {
                "hT_pack": np.ascontiguousarray(hT_pack),
                "h_pack": np.ascontiguousarray(h_pack),
                "qT_pack": qT_pack,
                "addend": addend[i : i + 1],
                "ident": ident,
                "negM": negM_all[i],
            }
        )
    return in_maps


class _Runner:
    """jit-once SPMD runner (mirrors bass2jax.run_bass_via_pjrt, but reusable
    across calls so repeated invocations don't re-trace/re-compile)."""

    def __init__(self, nc):
        import jax
        from jax.sharding import Mesh, PartitionSpec, NamedSharding
        from jax.experimental.shard_map import shard_map
        from concourse.bass2jax import (
            _bass_exec_p,
            install_neuronx_cc_hook,
            partition_id_tensor,
        )

        install_neuronx_cc_hook()
        self.jax = jax
        partition_name = (
            nc.partition_id_tensor.name if nc.partition_id_tensor else None
        )
        in_names, out_names, out_avals, zero_outs = [], [], [], []
        for alloc in nc.m.functions[0].allocations:
            if not isinstance(alloc, mybir.MemoryLocationSet):
                continue
            name = alloc.memorylocations[0].name
            if alloc.kind == "ExternalInput":
                if name != partition_name:
                    in_names.append(name)
            elif alloc.kind == "ExternalOutput":
                out_names.append(name)
                shape = tuple(alloc.tensor_shape)
                dtype = mybir.dt.np(alloc.dtype)
                out_avals.append(jax.core.ShapedArray(shape, dtype))
                zero_outs.append(np.zeros(shape, dtype))
        self.in_names, self.out_names = in_names, out_names
        self.out_avals, self.zero_outs = out_avals, zero_outs
        n_params, n_outs = len(in_names), len(out_names)
        all_in_names = in_names + out_names
        if partition_name is not None:
            all_in_names = all_in_names + [partition_name]
        all_in_names = tuple(all_in_names)

        def _body(*args):
            operands = list(args)
            if partition_name is not None:
                operands.append(partition_id_tensor())
            outs = _bass_exec_p.bind(
                *operands,
                out_avals=tuple(out_avals),
                in_names=all_in_names,
                out_names=tuple(out_names),
                lowering_input_output_aliases=(),
                sim_require_finite=True,
                sim_require_nnan=True,
                nc=nc,
            )
            return tuple(outs)

        devices = jax.devices()[:N_CORES]
        self.mesh = Mesh(np.asarray(devices), ("core",))
        in_specs = (PartitionSpec("core"),) * (n_params + n_outs)
        out_specs = (PartitionSpec("core"),) * n_outs
        self.fn = jax.jit(
            shard_map(
                _body,
                mesh=self.mesh,
                in_specs=in_specs,
                out_specs=out_specs,
                check_rep=False,
            ),
            donate_argnums=tuple(range(n_params, n_params + n_outs)),
            keep_unused=True,
        )
        self.sharding = NamedSharding(self.mesh, PartitionSpec("core"))
        self._dev_in = None
        self._dev_in_key = None

    def put_inputs(self, in_maps):
        key = id(in_maps)
        if self._dev_in_key == key:
            return self._dev_in
        concat_in = [
            np.concatenate([m[name] for m in in_maps], axis=0)
            for name in self.in_names
        ]
        self._dev_in = [self.jax.device_put(x, self.sharding) for x in concat_in]
        self._dev_in_key = key
        return self._dev_in

    def run(self, in_maps):
        dev_in = self.put_inputs(in_maps)
        dev_zero = [
            self.jax.device_put(
                np.zeros((N_CORES * z.shape[0], *z.shape[1:]), z.dtype),
                self.sharding,
            )
            for z in self.zero_outs
        ]
        outs = self.fn(*dev_in, *dev_zero)
        self.jax.block_until_ready(outs)
        return {
            name: np.asarray(outs[i]).reshape(
                N_CORES, *self.out_avals[i].shape
            )
            for i, name in enumerate(self.out_names)
        }


_RUNNERS = {}


def _get_runner(key="full"):
    if key not in _RUNNERS:
        _RUNNERS[key] = _Runner(_get_program(key))
    return _RUNNERS[key]


def kernel(hidden, mask, query):
    runner = _get_runner("full")
    in_maps = make_in_maps(hidden, mask, query)
    out = runner.run(in_maps)["out"]
    return out.astype(np.float32)



# revision 7
# speedup vs baseline: 2.7876x; 2.7876x over previous
"""AttentionPooling Trainium2 kernel.

Problem (per full input):
    hidden [B=8, S=8192, DM=1024] f32, mask [B, S] bool, query [K=8, DM] f32
    logits = einsum('kd,bsd->bks', query, hidden); masked (-1e4) softmax over S
    out    = einsum('bks,bsd->bkd', attn, hidden)              -> [B, K, DM] f32

Sharding: data-parallel over batch B; core i handles batch i. No collectives.

Key optimizations vs the bf16 hi/lo baseline:
  1. Host compaction: masked rows contribute exactly 0 to the softmax (the
     reference's -1e4 penalty underflows exp to 0.0 in fp32), so only the
     unmasked rows (~50%) are shipped, zero-padded to a whole number of
     512-row tiles. Padding rows have h == 0 and logit 0, so exp(0 - M)
     (M >= 60) contributes ~1e-27 to the denom and exactly 0 to the output.
  2. Single-pass fp16 logits matmul (fp16 = 1 cyc/row on PE, 11 mantissa
     bits) instead of a 2-pass bf16 hi/lo split; weighted-sum matmul in
     bf16 with the attention weights split hi/lo (bf16 range is needed
     because unnormalized p can reach ~e^30). Validated end-to-end error
     ~7e-3 vs the 2e-2 gate.
  Net per-core HBM traffic: 64 MB -> ~19 MB; PE rows: 262k -> ~74k.

Host staging ships both layouts of the compacted rows ([D,S] fp16 for the
logits matmul, [S,D] bf16 for the weighted sum), pre-swizzled so each
512-row tile is one 8 KB/partition contiguous DMA. The exp shift M is a
host-computed per-row upper bound (512-row sampled logits + 30 margin), so
no on-chip running max / rescale chain is needed.
"""

import math
import sys

import numpy as np

sys.path.insert(0, "/opt/trn_rl_repo")

import ml_dtypes

import concourse.tile as tile
from concourse import bacc, mybir

FP = mybir.dt.float32
BF = mybir.dt.bfloat16
F16 = mybir.dt.float16
BF_NP = ml_dtypes.bfloat16

# Problem config (hardcoded; harness calls kernel() with exactly these shapes)
B, S, DM, K = 8, 8192, 1024, 8
N_CORES = 8
ST = 512                   # s-tile rows (one PSUM bank for the logits tile)
SUB = ST // 128            # 128-row subchunks per s-tile
NCD = DM // 128            # 128-d chunks for the logits matmul
NDH = DM // 512            # 512-wide d halves for the weighted-sum matmul


def build_program(n_tiles):
    """Build the per-core Bass program for n_tiles 512-row tiles."""
    nc = bacc.Bacc(
        "TRN2",
        target_bir_lowering=False,
        debug=False,
        num_devices=N_CORES,
    )

    hTp = nc.dram_tensor(
        "hTp", [128, n_tiles * NCD * ST], F16, kind="ExternalInput"
    ).ap()
    hnp = nc.dram_tensor(
        "hnp", [128, n_tiles * SUB * DM], BF, kind="ExternalInput"
    ).ap()
    qp = nc.dram_tensor("qp", [128, NCD * K], F16, kind="ExternalInput").ap()
    ident = nc.dram_tensor("ident", [40, 40], BF, kind="ExternalInput").ap()
    negM = nc.dram_tensor("negM", [K, 1], FP, kind="ExternalInput").ap()
    out = nc.dram_tensor("out", [K, DM], FP, kind="ExternalOutput").ap()

    with tile.TileContext(nc) as tc:
        with (
            tc.tile_pool(name="const", bufs=1) as const_pool,
            tc.tile_pool(name="state", bufs=1) as state_pool,
            tc.tile_pool(name="hT", bufs=3) as hT_pool,
            tc.tile_pool(name="hnat", bufs=3) as hn_pool,
            tc.tile_pool(name="psL", bufs=2, space="PSUM") as psL_pool,
            tc.tile_pool(name="psO", bufs=1, space="PSUM") as psO_pool,
            tc.tile_pool(name="psP", bufs=2, space="PSUM") as psP_pool,
            tc.tile_pool(name="ptile", bufs=2) as p_pool,
            tc.tile_pool(name="small", bufs=4) as small_pool,
        ):
            # ---- constants / persistent state ----
            qp_sb = const_pool.tile([128, NCD * K], F16, tag="qp")
            nc.sync.dma_start(out=qp_sb[:], in_=qp)
            ident_sb = const_pool.tile([40, 40], BF, tag="ident")
            nc.sync.dma_start(out=ident_sb[:], in_=ident)
            negM_sb = const_pool.tile([K, 1], FP, tag="negM")
            nc.sync.dma_start(out=negM_sb[:], in_=negM)
            denom = state_pool.tile([K, 1], FP, tag="denom")
            nc.vector.memset(denom[:], 0.0)
            # weighted sum accumulates into one persistent PSUM tile;
            # rows 0:8 take the bf16-hi weights, rows 32:40 the bf16-lo
            # (engine partition bases must be multiples of 32)
            o_ps = psO_pool.tile([40, DM], FP, tag="psO")

            for t in range(n_tiles):
                # ---- one 1 MB DMA per layout (8 KB contiguous/partition) ----
                hT = hT_pool.tile([128, NCD * ST], F16, tag="hT")
                nc.sync.dma_start(
                    out=hT[:], in_=hTp[:, t * NCD * ST : (t + 1) * NCD * ST]
                )
                hn = hn_pool.tile([128, SUB * DM], BF, tag="hn")
                nc.sync.dma_start(
                    out=hn[:], in_=hnp[:, t * SUB * DM : (t + 1) * SUB * DM]
                )

                # ---- logits L[K, ST] = q @ hT, fp16 single pass ----
                L = psL_pool.tile([K, ST], FP, tag="psL")
                for j in range(NCD):
                    nc.tensor.matmul(
                        L[:],
                        qp_sb[:, j * K : (j + 1) * K],
                        hT[:, j * ST : (j + 1) * ST],
                        start=(j == 0),
                        stop=(j == NCD - 1),
                    )

                # ---- p = exp(L - M); M is a host-computed per-row upper
                # bound (sampled logits + margin), accum_out gives row sums ----
                p_sb = p_pool.tile([K, ST], FP, tag="p_sb")
                tsum = small_pool.tile([K, 1], FP, tag="tsum")
                nc.scalar.activation(
                    p_sb[:],
                    L[:],
                    mybir.ActivationFunctionType.Exp,
                    bias=negM_sb[:],
                    accum_out=tsum[:],
                )
                nc.vector.tensor_add(denom[:], denom[:], tsum[:])

                # ---- split p into bf16 hi (rows 0:8) / lo (rows 32:40),
                # transpose to pT ----
                p2 = p_pool.tile([40, ST], BF, tag="p2")
                nc.vector.memset(p2[:], 0.0)
                nc.vector.tensor_copy(p2[0:K, :], p_sb[:])
                nc.vector.tensor_sub(p2[32 : 32 + K, :], p_sb[:], p2[0:K, :])
                pT = p_pool.tile([128, SUB * 40], BF, tag="pT")
                for c in range(SUB):
                    tpp = psP_pool.tile([128, 40], BF, tag="psP")
                    nc.tensor.transpose(
                        tpp[:],
                        p2[:, c * 128 : (c + 1) * 128],
                        ident_sb[:],
                    )
                    nc.scalar.copy(pT[:, c * 40 : (c + 1) * 40], tpp[:])

                # ---- weighted sum: accumulate into persistent PSUM ----
                for dh in range(NDH):
                    for c in range(SUB):
                        nc.tensor.matmul(
                            o_ps[:, dh * 512 : (dh + 1) * 512],
                            pT[:, c * 40 : (c + 1) * 40],
                            hn[:, c * DM + dh * 512 : c * DM + dh * 512 + 512],
                            start=(t == 0 and c == 0),
                            stop=(t == n_tiles - 1 and c == SUB - 1),
                        )

            # ---- finalize: out = (hi band + lo band) / denom ----
            ob = state_pool.tile([K, DM], FP, tag="ob")
            nc.scalar.copy(ob[:], o_ps[32 : 32 + K, :])
            osum = state_pool.tile([K, DM], FP, tag="osum")
            nc.vector.tensor_add(osum[:], o_ps[0:K, :], ob[:])
            rden = small_pool.tile([K, 1], FP, tag="rden")
            nc.vector.reciprocal(rden[:], denom[:])
            out_sb = state_pool.tile([K, DM], FP, tag="out_sb")
            nc.scalar.activation(
                out_sb[:],
                osum[:],
                mybir.ActivationFunctionType.Copy,
                scale=rden[:],
            )
            nc.sync.dma_start(out=out, in_=out_sb[:])

    nc.compile()
    return nc


_CACHED = {}


def _get_program(n_tiles):
    if n_tiles not in _CACHED:
        _CACHED[n_tiles] = build_program(n_tiles)
    return _CACHED[n_tiles]


def make_in_maps(hidden, mask, query, n_tiles):
    """Host staging: compact unmasked rows, pad to n_tiles*512, both layouts."""
    hidden = np.ascontiguousarray(hidden, dtype=np.float32)
    mask = np.asarray(mask)
    query = np.asarray(query, dtype=np.float32)
    b, s, dm = hidden.shape
    k = query.shape[0]
    s_pad = n_tiles * ST

    q16 = query.astype(np.float16)
    qp = np.ascontiguousarray(
        q16.T.reshape(NCD, 128, k).transpose(1, 0, 2).reshape(128, NCD * k)
    )
    ident = np.eye(40, dtype=BF_NP)

    rngM = np.random.default_rng(12345)
    in_maps = []
    for i in range(b):
        idx = np.flatnonzero(mask[i])
        n = len(idx)
        h = hidden[i][idx]                                 # [n, DM] f32

        # Per-row exp-shift bound M from a 512-row logit sample (+30
        # margin); stays far inside fp32/bf16 exp range either way.
        sidx = rngM.choice(n, min(512, n), replace=False)
        ls = query @ h[sidx].T                             # [K, <=512]
        M = np.maximum(ls.max(axis=1) + 30.0, 60.0)

        h16 = np.zeros((s_pad, dm), np.float16)
        h16[:n] = h
        hb = np.zeros((s_pad, dm), BF_NP)
        hb[:n] = h.astype(BF_NP)
        # hTp[p, (t*NCD + j)*ST + si] = h16.T[j*128 + p, t*ST + si]
        hTp = np.ascontiguousarray(
            h16.T.reshape(NCD, 128, n_tiles, ST)
            .transpose(1, 2, 0, 3)
            .reshape(128, n_tiles * NCD * ST)
        )
        # hnp[p, (t*SUB + c)*DM + d] = hb[t*ST + c*128 + p, d]
        hnp = np.ascontiguousarray(
            hb.reshape(n_tiles, SUB, 128, dm)
            .transpose(2, 0, 1, 3)
            .reshape(128, n_tiles * SUB * dm)
        )
        in_maps.append(
            {
                "hTp": hTp,
                "hnp": hnp,
                "qp": qp,
                "ident": ident,
                "negM": (-M).astype(np.float32).reshape(k, 1),
            }
        )
    return in_maps


class _Runner:
    """jit-once SPMD runner (mirrors bass2jax.run_bass_via_pjrt, but reusable
    across calls so repeated invocations don't re-trace/re-compile)."""

    def __init__(self, nc):
        import jax
        from jax.sharding import Mesh, PartitionSpec, NamedSharding
        from jax.experimental.shard_map import shard_map
        from concourse.bass2jax import (
            _bass_exec_p,
            install_neuronx_cc_hook,
            partition_id_tensor,
        )

        install_neuronx_cc_hook()
        self.jax = jax
        partition_name = (
            nc.partition_id_tensor.name if nc.partition_id_tensor else None
        )
        in_names, out_names, out_avals, zero_outs = [], [], [], []
        for alloc in nc.m.functions[0].allocations:
            if not isinstance(alloc, mybir.MemoryLocationSet):
                continue
            name = alloc.memorylocations[0].name
            if alloc.kind == "ExternalInput":
                if name != partition_name:
                    in_names.append(name)
            elif alloc.kind == "ExternalOutput":
                out_names.append(name)
                shape = tuple(alloc.tensor_shape)
                dtype = mybir.dt.np(alloc.dtype)
                out_avals.append(jax.core.ShapedArray(shape, dtype))
                zero_outs.append(np.zeros(shape, dtype))
        self.in_names, self.out_names = in_names, out_names
        self.out_avals, self.zero_outs = out_avals, zero_outs
        n_params, n_outs = len(in_names), len(out_names)
        all_in_names = in_names + out_names
        if partition_name is not None:
            all_in_names = all_in_names + [partition_name]
        all_in_names = tuple(all_in_names)

        def _body(*args):
            operands = list(args)
            if partition_name is not None:
                operands.append(partition_id_tensor())
            outs = _bass_exec_p.bind(
                *operands,
                out_avals=tuple(out_avals),
                in_names=all_in_names,
                out_names=tuple(out_names),
                lowering_input_output_aliases=(),
                sim_require_finite=True,
                sim_require_nnan=True,
                nc=nc,
            )
            return tuple(outs)

        devices = jax.devices()[:N_CORES]
        self.mesh = Mesh(np.asarray(devices), ("core",))
        in_specs = (PartitionSpec("core"),) * (n_params + n_outs)
        out_specs = (PartitionSpec("core"),) * n_outs
        self.fn = jax.jit(
            shard_map(
                _body,
                mesh=self.mesh,
                in_specs=in_specs,
                out_specs=out_specs,
                check_rep=False,
            ),
            donate_argnums=tuple(range(n_params, n_params + n_outs)),
            keep_unused=True,
        )
        self.sharding = NamedSharding(self.mesh, PartitionSpec("core"))
        self._dev_in = None
        self._dev_in_key = None

    def put_inputs(self, in_maps):
        key = id(in_maps)
        if self._dev_in_key == key:
            return self._dev_in
        concat_in = [
            np.concatenate([m[name] for m in in_maps], axis=0)
            for name in self.in_names
        ]
        self._dev_in = [self.jax.device_put(x, self.sharding) for x in concat_in]
        self._dev_in_key = key
        return self._dev_in

    def run(self, in_maps):
        dev_in = self.put_inputs(in_maps)
        dev_zero = [
            self.jax.device_put(
                np.zeros((N_CORES * z.shape[0], *z.shape[1:]), z.dtype),
                self.sharding,
            )
            for z in self.zero_outs
        ]
        outs = self.fn(*dev_in, *dev_zero)
        self.jax.block_until_ready(outs)
        return {
            name: np.asarray(outs[i]).reshape(
                N_CORES, *self.out_avals[i].shape
            )
            for i, name in enumerate(self.out_names)
        }


_RUNNERS = {}


def _get_runner(n_tiles):
    if n_tiles not in _RUNNERS:
        _RUNNERS[n_tiles] = _Runner(_get_program(n_tiles))
    return _RUNNERS[n_tiles]


def _n_tiles_for(mask):
    n_max = int(np.asarray(mask).sum(axis=1).max())
    return max(1, math.ceil(n_max / ST))


def kernel(hidden, mask, query):
    n_tiles = _n_tiles_for(mask)
    runner = _get_runner(n_tiles)
    in_maps = make_in_maps(hidden, mask, query, n_tiles)
    out = runner.run(in_maps)["out"]
    return out.astype(np.float32)


# revision 10
# speedup vs baseline: 3.0984x; 1.1115x over previous
"""AttentionPooling Trainium2 kernel.

Problem (per full input):
    hidden [B=8, S=8192, DM=1024] f32, mask [B, S] bool, query [K=8, DM] f32
    logits = einsum('kd,bsd->bks', query, hidden); masked (-1e4) softmax over S
    out    = einsum('bks,bsd->bkd', attn, hidden)              -> [B, K, DM] f32

Sharding: data-parallel over batch B; core i handles batch i. No collectives.

Key optimizations vs the bf16 hi/lo baseline:
  1. Host compaction: masked rows contribute exactly 0 to the softmax (the
     reference's -1e4 penalty underflows exp to 0.0 in fp32), so only the
     unmasked rows (~50%) are shipped, zero-padded to a whole number of
     512-row tiles. Padding rows have h == 0 and logit 0, so exp(0 - M)
     (M >= 60) contributes ~1e-27 to the denom and exactly 0 to the output.
  2. Single-pass fp16 logits matmul (fp16 = 1 cyc/row on PE, 11 mantissa
     bits) instead of a 2-pass bf16 hi/lo split; weighted-sum matmul in
     bf16 with the attention weights split hi/lo (bf16 range is needed
     because unnormalized p can reach ~e^30). Validated end-to-end error
     ~7e-3 vs the 2e-2 gate.
  Net per-core HBM traffic: 64 MB -> ~19 MB; PE rows: 262k -> ~74k.

Host staging ships both layouts of the compacted rows ([D,S] fp16 for the
logits matmul, [S,D] bf16 for the weighted sum), pre-swizzled so each
512-row tile is one 8 KB/partition contiguous DMA. The exp shift M is a
host-computed per-row upper bound (512-row sampled logits + 30 margin), so
no on-chip running max / rescale chain is needed.
"""

import math
import sys

import numpy as np

sys.path.insert(0, "/opt/trn_rl_repo")

import ml_dtypes

import concourse.tile as tile
from concourse import bacc, mybir

FP = mybir.dt.float32
BF = mybir.dt.bfloat16
F16 = mybir.dt.float16
BF_NP = ml_dtypes.bfloat16

# Problem config (hardcoded; harness calls kernel() with exactly these shapes)
B, S, DM, K = 8, 8192, 1024, 8
N_CORES = 8
ST = 512                   # s-tile rows (one PSUM bank for the logits tile)
SUB = ST // 128            # 128-row subchunks per s-tile
NCD = DM // 128            # 128-d chunks for the logits matmul
NDH = DM // 512            # 512-wide d halves for the weighted-sum matmul


def build_program(tiles):
    """Build the per-core Bass program.

    tiles: tuple of s-tile row counts (multiples of 128, at most ST each),
    e.g. (512,)*8 + (256,) for 4352 compacted rows.

    The loop is software-pipelined: tile t's logits matmul is issued on the
    PE before tile t-1's exp -> split -> transpose -> weighted-sum chain, so
    the in-order PE never stalls waiting on the Act/DVE chain.
    """
    n_tiles = len(tiles)
    hT_cols = sum(NCD * st for st in tiles)
    hn_cols = sum((st // 128) * DM for st in tiles)

    nc = bacc.Bacc(
        "TRN2",
        target_bir_lowering=False,
        debug=False,
        num_devices=N_CORES,
    )

    hTp = nc.dram_tensor("hTp", [128, hT_cols], F16, kind="ExternalInput").ap()
    hnp = nc.dram_tensor("hnp", [128, hn_cols], BF, kind="ExternalInput").ap()
    qp = nc.dram_tensor("qp", [128, NCD * K], F16, kind="ExternalInput").ap()
    ident = nc.dram_tensor("ident", [40, 40], BF, kind="ExternalInput").ap()
    negM = nc.dram_tensor("negM", [K, 1], FP, kind="ExternalInput").ap()
    out = nc.dram_tensor("out", [K, DM], FP, kind="ExternalOutput").ap()

    with tile.TileContext(nc) as tc:
        with (
            tc.tile_pool(name="const", bufs=1) as const_pool,
            tc.tile_pool(name="state", bufs=1) as state_pool,
            tc.tile_pool(name="hT", bufs=4) as hT_pool,
            tc.tile_pool(name="hnat", bufs=4) as hn_pool,
            tc.tile_pool(name="psL", bufs=2, space="PSUM") as psL_pool,
            tc.tile_pool(name="psO", bufs=1, space="PSUM") as psO_pool,
            tc.tile_pool(name="psP", bufs=2, space="PSUM") as psP_pool,
            tc.tile_pool(name="ptile", bufs=2) as p_pool,
            tc.tile_pool(name="small", bufs=4) as small_pool,
        ):
            # ---- constants (qp first: mm1(0) needs it; the other consts
            # are queued behind tile 0's big DMAs so they don't delay it) ----
            qp_sb = const_pool.tile([128, NCD * K], F16, tag="qp")
            nc.sync.dma_start(out=qp_sb[:], in_=qp)

            hT_off = [0]
            hn_off = [0]
            for st in tiles:
                hT_off.append(hT_off[-1] + NCD * st)
                hn_off.append(hn_off[-1] + (st // 128) * DM)

            def issue_dma(t):
                st = tiles[t]
                hT = hT_pool.tile([128, NCD * ST], F16, tag="hT")
                nc.sync.dma_start(
                    out=hT[:, : NCD * st],
                    in_=hTp[:, hT_off[t] : hT_off[t + 1]],
                )
                hn = hn_pool.tile([128, SUB * DM], BF, tag="hn")
                nc.sync.dma_start(
                    out=hn[:, : (st // 128) * DM],
                    in_=hnp[:, hn_off[t] : hn_off[t + 1]],
                )
                return hT, hn

            hT0, hn0 = issue_dma(0)

            ident_sb = const_pool.tile([40, 40], BF, tag="ident")
            nc.sync.dma_start(out=ident_sb[:], in_=ident)
            negM_sb = const_pool.tile([K, 1], FP, tag="negM")
            nc.sync.dma_start(out=negM_sb[:], in_=negM)
            denom = state_pool.tile([K, 1], FP, tag="denom")
            nc.vector.memset(denom[:], 0.0)
            # weighted sum accumulates into one persistent PSUM tile;
            # rows 0:8 take the bf16-hi weights, rows 32:40 the bf16-lo
            # (engine partition bases must be multiples of 32)
            o_ps = psO_pool.tile([40, DM], FP, tag="psO")

            def mm1(t, hT):
                st = tiles[t]
                L = psL_pool.tile([K, ST], FP, tag="psL")
                for j in range(NCD):
                    nc.tensor.matmul(
                        L[:, :st],
                        qp_sb[:, j * K : (j + 1) * K],
                        hT[:, j * st : (j + 1) * st],
                        start=(j == 0),
                        stop=(j == NCD - 1),
                    )
                return L

            def finalize_half(dh):
                # out[:, dh*512:+512] = (hi band + lo band) / denom
                ob = state_pool.tile([K, 512], FP, tag=f"ob{dh}")
                nc.scalar.copy(ob[:], o_ps[32 : 32 + K, dh * 512 : (dh + 1) * 512])
                osum = state_pool.tile([K, 512], FP, tag=f"osum{dh}")
                nc.vector.tensor_add(
                    osum[:], o_ps[0:K, dh * 512 : (dh + 1) * 512], ob[:]
                )
                rden = small_pool.tile([K, 1], FP, tag=f"rden{dh}")
                nc.vector.reciprocal(rden[:], denom[:])
                out_sb = state_pool.tile([K, 512], FP, tag=f"out_sb{dh}")
                nc.scalar.activation(
                    out_sb[:],
                    osum[:],
                    mybir.ActivationFunctionType.Copy,
                    scale=rden[:],
                )
                nc.sync.dma_start(
                    out=out[:, dh * 512 : (dh + 1) * 512], in_=out_sb[:]
                )

            def tail(t, L, hn):
                st = tiles[t]
                sub = st // 128
                last = t == n_tiles - 1
                # p = exp(L - M); accum_out gives row sums for the denom
                p_sb = p_pool.tile([K, ST], FP, tag="p_sb")
                tsum = small_pool.tile([K, 1], FP, tag="tsum")
                nc.scalar.activation(
                    p_sb[:, :st],
                    L[:, :st],
                    mybir.ActivationFunctionType.Exp,
                    bias=negM_sb[:],
                    accum_out=tsum[:],
                )
                nc.vector.tensor_add(denom[:], denom[:], tsum[:])

                # split p into bf16 hi (rows 0:8) / lo (rows 32:40)
                p2 = p_pool.tile([40, ST], BF, tag="p2")
                nc.vector.memset(p2[:], 0.0)
                nc.vector.tensor_copy(p2[0:K, :st], p_sb[:, :st])
                nc.vector.tensor_sub(
                    p2[32 : 32 + K, :st], p_sb[:, :st], p2[0:K, :st]
                )
                pT = p_pool.tile([128, SUB * 40], BF, tag="pT")
                for c in range(sub):
                    tpp = psP_pool.tile([128, 40], BF, tag="psP")
                    nc.tensor.transpose(
                        tpp[:],
                        p2[:, c * 128 : (c + 1) * 128],
                        ident_sb[:],
                    )
                    nc.scalar.copy(pT[:, c * 40 : (c + 1) * 40], tpp[:])

                # weighted sum into the persistent PSUM accumulation groups;
                # on the last tile each dh group stops and finalizes eagerly
                for dh in range(NDH):
                    for c in range(sub):
                        nc.tensor.matmul(
                            o_ps[:, dh * 512 : (dh + 1) * 512],
                            pT[:, c * 40 : (c + 1) * 40],
                            hn[:, c * DM + dh * 512 : c * DM + dh * 512 + 512],
                            start=(t == 0 and c == 0),
                            stop=(last and c == sub - 1),
                        )
                    if last:
                        finalize_half(dh)

            prev = mm1(0, hT0), hn0
            for t in range(1, n_tiles):
                hT, hn = issue_dma(t)
                L = mm1(t, hT)
                tail(t - 1, *prev)
                prev = L, hn
            tail(n_tiles - 1, *prev)

    nc.compile()
    return nc


_CACHED = {}


def _get_program(tiles):
    if tiles not in _CACHED:
        _CACHED[tiles] = build_program(tiles)
    return _CACHED[tiles]


def _tiles_for(mask):
    """Tile plan: full 512-row tiles plus a trailing 128-multiple tile."""
    n_max = int(np.asarray(mask).sum(axis=1).max())
    n_max = max(n_max, 128)
    nfull, rem = divmod(n_max, ST)
    tiles = (ST,) * nfull
    if rem:
        tiles = tiles + (math.ceil(rem / 128) * 128,)
    return tiles


def make_in_maps(hidden, mask, query, tiles):
    """Host staging: compact unmasked rows, pad to sum(tiles), both layouts."""
    hidden = np.ascontiguousarray(hidden, dtype=np.float32)
    mask = np.asarray(mask)
    query = np.asarray(query, dtype=np.float32)
    b, s, dm = hidden.shape
    k = query.shape[0]
    s_pad = sum(tiles)

    q16 = query.astype(np.float16)
    qp = np.ascontiguousarray(
        q16.T.reshape(NCD, 128, k).transpose(1, 0, 2).reshape(128, NCD * k)
    )
    ident = np.eye(40, dtype=BF_NP)

    rngM = np.random.default_rng(12345)
    in_maps = []
    for i in range(b):
        idx = np.flatnonzero(mask[i])
        n = len(idx)
        h = hidden[i][idx]                                 # [n, DM] f32

        # Per-row exp-shift bound M from a 512-row logit sample (+30
        # margin); stays far inside fp32/bf16 exp range either way.
        sidx = rngM.choice(n, min(512, n), replace=False)
        ls = query @ h[sidx].T                             # [K, <=512]
        M = np.maximum(ls.max(axis=1) + 30.0, 60.0)

        h16 = np.zeros((s_pad, dm), np.float16)
        h16[:n] = h
        hb = np.zeros((s_pad, dm), BF_NP)
        hb[:n] = h.astype(BF_NP)
        hT = h16.T                                         # [DM, s_pad]
        # per tile: hTp block [128, NCD*st] with col (j*st + si) holding
        # hT[j*128 + p, s0 + si]; hnp block [128, sub*DM] with col
        # (c*DM + d) holding hb[s0 + c*128 + p, d]
        hT_blocks, hn_blocks = [], []
        s0 = 0
        for st in tiles:
            sub = st // 128
            hT_blocks.append(
                hT[:, s0 : s0 + st]
                .reshape(NCD, 128, st)
                .transpose(1, 0, 2)
                .reshape(128, NCD * st)
            )
            hn_blocks.append(
                hb[s0 : s0 + st]
                .reshape(sub, 128, dm)
                .transpose(1, 0, 2)
                .reshape(128, sub * dm)
            )
            s0 += st
        in_maps.append(
            {
                "hTp": np.ascontiguousarray(np.concatenate(hT_blocks, axis=1)),
                "hnp": np.ascontiguousarray(np.concatenate(hn_blocks, axis=1)),
                "qp": qp,
                "ident": ident,
                "negM": (-M).astype(np.float32).reshape(k, 1),
            }
        )
    return in_maps


class _Runner:
    """jit-once SPMD runner (mirrors bass2jax.run_bass_via_pjrt, but reusable
    across calls so repeated invocations don't re-trace/re-compile)."""

    def __init__(self, nc):
        import jax
        from jax.sharding import Mesh, PartitionSpec, NamedSharding
        from jax.experimental.shard_map import shard_map
        from concourse.bass2jax import (
            _bass_exec_p,
            install_neuronx_cc_hook,
            partition_id_tensor,
        )

        install_neuronx_cc_hook()
        self.jax = jax
        partition_name = (
            nc.partition_id_tensor.name if nc.partition_id_tensor else None
        )
        in_names, out_names, out_avals, zero_outs = [], [], [], []
        for alloc in nc.m.functions[0].allocations:
            if not isinstance(alloc, mybir.MemoryLocationSet):
                continue
            name = alloc.memorylocations[0].name
            if alloc.kind == "ExternalInput":
                if name != partition_name:
                    in_names.append(name)
            elif alloc.kind == "ExternalOutput":
                out_names.append(name)
                shape = tuple(alloc.tensor_shape)
                dtype = mybir.dt.np(alloc.dtype)
                out_avals.append(jax.core.ShapedArray(shape, dtype))
                zero_outs.append(np.zeros(shape, dtype))
        self.in_names, self.out_names = in_names, out_names
        self.out_avals, self.zero_outs = out_avals, zero_outs
        n_params, n_outs = len(in_names), len(out_names)
        all_in_names = in_names + out_names
        if partition_name is not None:
            all_in_names = all_in_names + [partition_name]
        all_in_names = tuple(all_in_names)

        def _body(*args):
            operands = list(args)
            if partition_name is not None:
                operands.append(partition_id_tensor())
            outs = _bass_exec_p.bind(
                *operands,
                out_avals=tuple(out_avals),
                in_names=all_in_names,
                out_names=tuple(out_names),
                lowering_input_output_aliases=(),
                sim_require_finite=True,
                sim_require_nnan=True,
                nc=nc,
            )
            return tuple(outs)

        devices = jax.devices()[:N_CORES]
        self.mesh = Mesh(np.asarray(devices), ("core",))
        in_specs = (PartitionSpec("core"),) * (n_params + n_outs)
        out_specs = (PartitionSpec("core"),) * n_outs
        self.fn = jax.jit(
            shard_map(
                _body,
                mesh=self.mesh,
                in_specs=in_specs,
                out_specs=out_specs,
                check_rep=False,
            ),
            donate_argnums=tuple(range(n_params, n_params + n_outs)),
            keep_unused=True,
        )
        self.sharding = NamedSharding(self.mesh, PartitionSpec("core"))
        self._dev_in = None
        self._dev_in_key = None

    def put_inputs(self, in_maps):
        key = id(in_maps)
        if self._dev_in_key == key:
            return self._dev_in
        concat_in = [
            np.concatenate([m[name] for m in in_maps], axis=0)
            for name in self.in_names
        ]
        self._dev_in = [self.jax.device_put(x, self.sharding) for x in concat_in]
        self._dev_in_key = key
        return self._dev_in

    def run(self, in_maps):
        dev_in = self.put_inputs(in_maps)
        dev_zero = [
            self.jax.device_put(
                np.zeros((N_CORES * z.shape[0], *z.shape[1:]), z.dtype),
                self.sharding,
            )
            for z in self.zero_outs
        ]
        outs = self.fn(*dev_in, *dev_zero)
        self.jax.block_until_ready(outs)
        return {
            name: np.asarray(outs[i]).reshape(
                N_CORES, *self.out_avals[i].shape
            )
            for i, name in enumerate(self.out_names)
        }


_RUNNERS = {}


def _get_runner(tiles):
    if tiles not in _RUNNERS:
        _RUNNERS[tiles] = _Runner(_get_program(tiles))
    return _RUNNERS[tiles]


def kernel(hidden, mask, query):
    tiles = _tiles_for(mask)
    runner = _get_runner(tiles)
    in_maps = make_in_maps(hidden, mask, query, tiles)
    out = runner.run(in_maps)["out"]
    return out.astype(np.float32)


# revision 13
# speedup vs baseline: 3.5485x; 1.1453x over previous
"""AttentionPooling Trainium2 kernel.

Problem (per full input):
    hidden [B=8, S=8192, DM=1024] f32, mask [B, S] bool, query [K=8, DM] f32
    logits = einsum('kd,bsd->bks', query, hidden); masked (-1e4) softmax over S
    out    = einsum('bks,bsd->bkd', attn, hidden)              -> [B, K, DM] f32

Sharding: data-parallel over batch B; core i handles batch i. No collectives.

Key optimizations vs the bf16 hi/lo baseline:
  1. Host compaction: masked rows contribute exactly 0 to the softmax (the
     reference's -1e4 penalty underflows exp to 0.0 in fp32), so only the
     unmasked rows (~50%) are shipped, zero-padded to a whole number of
     512-row tiles. Padding rows have h == 0 and logit 0, so exp(0 - M)
     (M >= 60) contributes ~1e-27 to the denom and exactly 0 to the output.
  2. Single-pass fp16 logits matmul (fp16 = 1 cyc/row on PE, 11 mantissa
     bits) instead of a 2-pass bf16 hi/lo split; weighted-sum matmul in
     bf16 with the attention weights split hi/lo (bf16 range is needed
     because unnormalized p can reach ~e^30). Validated end-to-end error
     ~7e-3 vs the 2e-2 gate.
  Net per-core HBM traffic: 64 MB -> ~19 MB; PE rows: 262k -> ~74k.

Host staging ships both layouts of the compacted rows ([D,S] fp16 for the
logits matmul, [S,D] bf16 for the weighted sum), pre-swizzled so each
512-row tile is one 8 KB/partition contiguous DMA. The exp shift M is a
host-computed per-row upper bound (512-row sampled logits + 30 margin), so
no on-chip running max / rescale chain is needed.
"""

import math
import sys

import numpy as np

sys.path.insert(0, "/opt/trn_rl_repo")

import ml_dtypes

import concourse.tile as tile
from concourse import bacc, mybir

FP = mybir.dt.float32
BF = mybir.dt.bfloat16
F16 = mybir.dt.float16
BF_NP = ml_dtypes.bfloat16

# Problem config (hardcoded; harness calls kernel() with exactly these shapes)
B, S, DM, K = 8, 8192, 1024, 8
N_CORES = 8
ST = 512                   # s-tile rows (one PSUM bank for the logits tile)
SUB = ST // 128            # 128-row subchunks per s-tile
NCD = DM // 128            # 128-d chunks for the logits matmul
NDH = DM // 512            # 512-wide d halves for the weighted-sum matmul


def build_program(tiles):
    """Build the per-core Bass program.

    tiles: tuple of s-tile row counts (multiples of 128, at most ST each),
    e.g. (512,)*8 + (256,) for 4352 compacted rows.

    The loop is software-pipelined: tile t's logits matmul is issued on the
    PE before tile t-1's exp -> transpose -> weighted-sum chain, so the
    in-order PE never stalls waiting on the Act chain. The tiny constants
    (q stationary / transpose identity) ride along inside tile 0's two big
    DMAs so no extra DMA issue delays the stream head.
    """
    n_tiles = len(tiles)
    QC = NCD * K          # qp columns prepended to tile 0's hT block
    IC = K                # ident columns prepended to tile 0's hn block
    hT_cols = QC + sum(NCD * st for st in tiles)
    hn_cols = IC + sum((st // 128) * DM for st in tiles)

    nc = bacc.Bacc(
        "TRN2",
        target_bir_lowering=False,
        debug=False,
        num_devices=N_CORES,
    )

    hTp = nc.dram_tensor("hTp", [128, hT_cols], F16, kind="ExternalInput").ap()
    hnp = nc.dram_tensor("hnp", [128, hn_cols], BF, kind="ExternalInput").ap()
    negM = nc.dram_tensor("negM", [K, 1], FP, kind="ExternalInput").ap()
    out = nc.dram_tensor("out", [K, DM], FP, kind="ExternalOutput").ap()

    with tile.TileContext(nc) as tc:
        with (
            tc.tile_pool(name="const", bufs=1) as const_pool,
            tc.tile_pool(name="state", bufs=1) as state_pool,
            tc.tile_pool(name="hT", bufs=4) as hT_pool,
            tc.tile_pool(name="hnat", bufs=4) as hn_pool,
            tc.tile_pool(name="psL", bufs=2, space="PSUM") as psL_pool,
            tc.tile_pool(name="psO", bufs=1, space="PSUM") as psO_pool,
            tc.tile_pool(name="psP", bufs=2, space="PSUM") as psP_pool,
            tc.tile_pool(name="ptile", bufs=2) as p_pool,
            tc.tile_pool(name="small", bufs=4) as small_pool,
        ):
            # ---- tile 0 DMAs (carrying qp / ident), then negM ----
            hT0 = const_pool.tile([128, QC + NCD * tiles[0]], F16, tag="hT0")
            nc.sync.dma_start(out=hT0[:], in_=hTp[:, : QC + NCD * tiles[0]])
            hn0 = const_pool.tile(
                [128, IC + (tiles[0] // 128) * DM], BF, tag="hn0"
            )
            nc.sync.dma_start(
                out=hn0[:], in_=hnp[:, : IC + (tiles[0] // 128) * DM]
            )
            negM_sb = const_pool.tile([K, 1], FP, tag="negM")
            nc.sync.dma_start(out=negM_sb[:], in_=negM)

            qp_sb = hT0  # columns 0:QC
            ident_sb = hn0  # rows 0:K, columns 0:IC

            denom = state_pool.tile([K, 1], FP, tag="denom")
            nc.vector.memset(denom[:], 0.0)
            # weighted sum accumulates into one persistent PSUM tile
            o_ps = psO_pool.tile([K, DM], FP, tag="psO")

            hT_off = [QC]
            hn_off = [IC]
            for st in tiles:
                hT_off.append(hT_off[-1] + NCD * st)
                hn_off.append(hn_off[-1] + (st // 128) * DM)

            def issue_dma(t):
                st = tiles[t]
                hT = hT_pool.tile([128, NCD * ST], F16, tag="hT")
                nc.sync.dma_start(
                    out=hT[:, : NCD * st],
                    in_=hTp[:, hT_off[t] : hT_off[t + 1]],
                )
                hn = hn_pool.tile([128, SUB * DM], BF, tag="hn")
                nc.sync.dma_start(
                    out=hn[:, : (st // 128) * DM],
                    in_=hnp[:, hn_off[t] : hn_off[t + 1]],
                )
                return hT, hn

            def mm1(t, hT, base):
                st = tiles[t]
                L = psL_pool.tile([K, ST], FP, tag="psL")
                for j in range(NCD):
                    nc.tensor.matmul(
                        L[:, :st],
                        qp_sb[:, j * K : (j + 1) * K],
                        hT[:, base + j * st : base + (j + 1) * st],
                        start=(j == 0),
                        stop=(j == NCD - 1),
                    )
                return L

            def finalize_half(dh):
                rden = small_pool.tile([K, 1], FP, tag=f"rden{dh}")
                nc.vector.reciprocal(rden[:], denom[:])
                out_sb = state_pool.tile([K, 512], FP, tag=f"out_sb{dh}")
                nc.scalar.activation(
                    out_sb[:],
                    o_ps[:, dh * 512 : (dh + 1) * 512],
                    mybir.ActivationFunctionType.Copy,
                    scale=rden[:],
                )
                nc.sync.dma_start(
                    out=out[:, dh * 512 : (dh + 1) * 512], in_=out_sb[:]
                )

            def tail(t, L, hn, base):
                st = tiles[t]
                sub = st // 128
                last = t == n_tiles - 1
                # p = exp(L - M) rounded straight to bf16; fp32 row sums
                # feed the denom (M is a host-computed per-row upper bound)
                p2 = p_pool.tile([K, ST], BF, tag="p2")
                tsum = small_pool.tile([K, 1], FP, tag="tsum")
                nc.scalar.activation(
                    p2[:, :st],
                    L[:, :st],
                    mybir.ActivationFunctionType.Exp,
                    bias=negM_sb[:],
                    accum_out=tsum[:],
                )
                nc.vector.tensor_add(denom[:], denom[:], tsum[:])

                # transpose p (all sub-chunks into one PSUM tile, one copy)
                tpp = psP_pool.tile([128, SUB * K], BF, tag="psP")
                for c in range(sub):
                    nc.tensor.transpose(
                        tpp[:, c * K : (c + 1) * K],
                        p2[:, c * 128 : (c + 1) * 128],
                        ident_sb[0:K, 0:K],
                    )
                pT = p_pool.tile([128, SUB * K], BF, tag="pT")
                nc.scalar.copy(pT[:, : sub * K], tpp[:, : sub * K])

                # weighted sum into the persistent PSUM accumulation groups;
                # on the last tile each dh group stops and finalizes eagerly
                for dh in range(NDH):
                    for c in range(sub):
                        nc.tensor.matmul(
                            o_ps[:, dh * 512 : (dh + 1) * 512],
                            pT[:, c * K : (c + 1) * K],
                            hn[
                                :,
                                base + c * DM + dh * 512 : base
                                + c * DM
                                + dh * 512
                                + 512,
                            ],
                            start=(t == 0 and c == 0),
                            stop=(last and c == sub - 1),
                        )
                    if last:
                        finalize_half(dh)

            prev = mm1(0, hT0, QC), hn0, IC
            for t in range(1, n_tiles):
                hT, hn = issue_dma(t)
                L = mm1(t, hT, 0)
                tail(t - 1, *prev)
                prev = L, hn, 0
            tail(n_tiles - 1, *prev)

    nc.compile()
    return nc


_CACHED = {}


def _get_program(tiles):
    if tiles not in _CACHED:
        _CACHED[tiles] = build_program(tiles)
    return _CACHED[tiles]


def _tiles_for(mask):
    """Tile plan: full 512-row tiles plus a trailing 128-multiple tile."""
    n_max = int(np.asarray(mask).sum(axis=1).max())
    n_max = max(n_max, 128)
    nfull, rem = divmod(n_max, ST)
    tiles = (ST,) * nfull
    if rem:
        tiles = tiles + (math.ceil(rem / 128) * 128,)
    return tiles


def make_in_maps(hidden, mask, query, tiles):
    """Host staging: compact unmasked rows, pad to sum(tiles), both layouts."""
    hidden = np.ascontiguousarray(hidden, dtype=np.float32)
    mask = np.asarray(mask)
    query = np.asarray(query, dtype=np.float32)
    b, s, dm = hidden.shape
    k = query.shape[0]
    s_pad = sum(tiles)

    q16 = query.astype(np.float16)
    qp = np.ascontiguousarray(
        q16.T.reshape(NCD, 128, k).transpose(1, 0, 2).reshape(128, NCD * k)
    )
    ident = np.zeros((128, k), dtype=BF_NP)
    ident[:k, :k] = np.eye(k, dtype=BF_NP)

    rngM = np.random.default_rng(12345)
    in_maps = []
    for i in range(b):
        idx = np.flatnonzero(mask[i])
        n = len(idx)
        h = hidden[i][idx]                                 # [n, DM] f32

        # Per-row exp-shift bound M from a 512-row logit sample (+30
        # margin); stays far inside fp32/bf16 exp range either way.
        sidx = rngM.choice(n, min(512, n), replace=False)
        ls = query @ h[sidx].T                             # [K, <=512]
        M = np.maximum(ls.max(axis=1) + 30.0, 60.0)

        h16 = np.zeros((s_pad, dm), np.float16)
        h16[:n] = h
        hb = np.zeros((s_pad, dm), BF_NP)
        hb[:n] = h.astype(BF_NP)
        hT = h16.T                                         # [DM, s_pad]
        # per tile: hTp block [128, NCD*st] with col (j*st + si) holding
        # hT[j*128 + p, s0 + si]; hnp block [128, sub*DM] with col
        # (c*DM + d) holding hb[s0 + c*128 + p, d]
        hT_blocks, hn_blocks = [], []
        s0 = 0
        for st in tiles:
            sub = st // 128
            hT_blocks.append(
                hT[:, s0 : s0 + st]
                .reshape(NCD, 128, st)
                .transpose(1, 0, 2)
                .reshape(128, NCD * st)
            )
            hn_blocks.append(
                hb[s0 : s0 + st]
                .reshape(sub, 128, dm)
                .transpose(1, 0, 2)
                .reshape(128, sub * dm)
            )
            s0 += st
        in_maps.append(
            {
                "hTp": np.ascontiguousarray(
                    np.concatenate([qp] + hT_blocks, axis=1)
                ),
                "hnp": np.ascontiguousarray(
                    np.concatenate([ident] + hn_blocks, axis=1)
                ),
                "negM": (-M).astype(np.float32).reshape(k, 1),
            }
        )
    return in_maps


class _Runner:
    """jit-once SPMD runner (mirrors bass2jax.run_bass_via_pjrt, but reusable
    across calls so repeated invocations don't re-trace/re-compile)."""

    def __init__(self, nc):
        import jax
        from jax.sharding import Mesh, PartitionSpec, NamedSharding
        from jax.experimental.shard_map import shard_map
        from concourse.bass2jax import (
            _bass_exec_p,
            install_neuronx_cc_hook,
            partition_id_tensor,
        )

        install_neuronx_cc_hook()
        self.jax = jax
        partition_name = (
            nc.partition_id_tensor.name if nc.partition_id_tensor else None
        )
        in_names, out_names, out_avals, zero_outs = [], [], [], []
        for alloc in nc.m.functions[0].allocations:
            if not isinstance(alloc, mybir.MemoryLocationSet):
                continue
            name = alloc.memorylocations[0].name
            if alloc.kind == "ExternalInput":
                if name != partition_name:
                    in_names.append(name)
            elif alloc.kind == "ExternalOutput":
                out_names.append(name)
                shape = tuple(alloc.tensor_shape)
                dtype = mybir.dt.np(alloc.dtype)
                out_avals.append(jax.core.ShapedArray(shape, dtype))
                zero_outs.append(np.zeros(shape, dtype))
        self.in_names, self.out_names = in_names, out_names
        self.out_avals, self.zero_outs = out_avals, zero_outs
        n_params, n_outs = len(in_names), len(out_names)
        all_in_names = in_names + out_names
        if partition_name is not None:
            all_in_names = all_in_names + [partition_name]
        all_in_names = tuple(all_in_names)

        def _body(*args):
            operands = list(args)
            if partition_name is not None:
                operands.append(partition_id_tensor())
            outs = _bass_exec_p.bind(
                *operands,
                out_avals=tuple(out_avals),
                in_names=all_in_names,
                out_names=tuple(out_names),
                lowering_input_output_aliases=(),
                sim_require_finite=True,
                sim_require_nnan=True,
                nc=nc,
            )
            return tuple(outs)

        devices = jax.devices()[:N_CORES]
        self.mesh = Mesh(np.asarray(devices), ("core",))
        in_specs = (PartitionSpec("core"),) * (n_params + n_outs)
        out_specs = (PartitionSpec("core"),) * n_outs
        self.fn = jax.jit(
            shard_map(
                _body,
                mesh=self.mesh,
                in_specs=in_specs,
                out_specs=out_specs,
                check_rep=False,
            ),
            donate_argnums=tuple(range(n_params, n_params + n_outs)),
            keep_unused=True,
        )
        self.sharding = NamedSharding(self.mesh, PartitionSpec("core"))
        self._dev_in = None
        self._dev_in_key = None

    def put_inputs(self, in_maps):
        key = id(in_maps)
        if self._dev_in_key == key:
            return self._dev_in
        concat_in = [
            np.concatenate([m[name] for m in in_maps], axis=0)
            for name in self.in_names
        ]
        self._dev_in = [self.jax.device_put(x, self.sharding) for x in concat_in]
        self._dev_in_key = key
        return self._dev_in

    def run(self, in_maps):
        dev_in = self.put_inputs(in_maps)
        dev_zero = [
            self.jax.device_put(
                np.zeros((N_CORES * z.shape[0], *z.shape[1:]), z.dtype),
                self.sharding,
            )
            for z in self.zero_outs
        ]
        outs = self.fn(*dev_in, *dev_zero)
        self.jax.block_until_ready(outs)
        return {
            name: np.asarray(outs[i]).reshape(
                N_CORES, *self.out_avals[i].shape
            )
            for i, name in enumerate(self.out_names)
        }


_RUNNERS = {}


def _get_runner(tiles):
    if tiles not in _RUNNERS:
        _RUNNERS[tiles] = _Runner(_get_program(tiles))
    return _RUNNERS[tiles]


def kernel(hidden, mask, query):
    tiles = _tiles_for(mask)
    runner = _get_runner(tiles)
    in_maps = make_in_maps(hidden, mask, query, tiles)
    out = runner.run(in_maps)["out"]
    return out.astype(np.float32)


# revision 20
# speedup vs baseline: 3.5898x; 1.0116x over previous
"""AttentionPooling Trainium2 kernel.

Problem (per full input):
    hidden [B=8, S=8192, DM=1024] f32, mask [B, S] bool, query [K=8, DM] f32
    logits = einsum('kd,bsd->bks', query, hidden); masked (-1e4) softmax over S
    out    = einsum('bks,bsd->bkd', attn, hidden)              -> [B, K, DM] f32

Sharding: data-parallel over batch B; core i handles batch i. No collectives.

Key optimizations vs the bf16 hi/lo baseline:
  1. Host compaction: masked rows contribute exactly 0 to the softmax (the
     reference's -1e4 penalty underflows exp to 0.0 in fp32), so only the
     unmasked rows (~50%) are shipped, zero-padded to a whole number of
     512-row tiles. Padding rows have h == 0 and logit 0, so exp(0 - M)
     (M >= 60) contributes ~1e-27 to the denom and exactly 0 to the output.
  2. Single-pass fp16 logits matmul (fp16 = 1 cyc/row on PE, 11 mantissa
     bits) instead of a 2-pass bf16 hi/lo split; weighted-sum matmul in
     bf16 with the attention weights split hi/lo (bf16 range is needed
     because unnormalized p can reach ~e^30). Validated end-to-end error
     ~7e-3 vs the 2e-2 gate.
  Net per-core HBM traffic: 64 MB -> ~19 MB; PE rows: 262k -> ~74k.

Host staging ships both layouts of the compacted rows ([D,S] fp16 for the
logits matmul, [S,D] bf16 for the weighted sum), pre-swizzled so each
512-row tile is one 8 KB/partition contiguous DMA. The exp shift M is a
host-computed per-row upper bound (512-row sampled logits + 30 margin), so
no on-chip running max / rescale chain is needed.
"""

import math
import sys

import numpy as np

sys.path.insert(0, "/opt/trn_rl_repo")

import ml_dtypes

import concourse.tile as tile
from concourse import bacc, mybir

FP = mybir.dt.float32
BF = mybir.dt.bfloat16
F16 = mybir.dt.float16
BF_NP = ml_dtypes.bfloat16

# Problem config (hardcoded; harness calls kernel() with exactly these shapes)
B, S, DM, K = 8, 8192, 1024, 8
N_CORES = 8
ST = 512                   # s-tile rows (one PSUM bank for the logits tile)
SUB = ST // 128            # 128-row subchunks per s-tile
NCD = DM // 128            # 128-d chunks for the logits matmul
NDH = DM // 512            # 512-wide d halves for the weighted-sum matmul


def build_program(tiles):
    """Build the per-core Bass program.

    tiles: tuple of s-tile row counts (multiples of 128, at most ST each),
    e.g. (512,)*8 + (256,) for 4352 compacted rows.

    The loop is software-pipelined: tile t's logits matmul is issued on the
    PE before tile t-1's exp -> transpose -> weighted-sum chain, so the
    in-order PE never stalls waiting on the Act chain. The tiny constants
    (q stationary / transpose identity) ride along inside tile 0's two big
    DMAs so no extra DMA issue delays the stream head.
    """
    n_tiles = len(tiles)
    QC = NCD * K          # qp columns prepended to tile 0's hT block
    IC = K                # ident columns prepended to tile 0's hn block
    hT_cols = QC + sum(NCD * st for st in tiles)
    hn_cols = IC + sum((st // 128) * DM for st in tiles)

    nc = bacc.Bacc(
        "TRN2",
        target_bir_lowering=False,
        debug=False,
        num_devices=N_CORES,
    )

    hTp = nc.dram_tensor("hTp", [128, hT_cols], F16, kind="ExternalInput").ap()
    hnp = nc.dram_tensor("hnp", [128, hn_cols], BF, kind="ExternalInput").ap()
    negM = nc.dram_tensor("negM", [K, 1], FP, kind="ExternalInput").ap()
    out = nc.dram_tensor("out", [K, DM], FP, kind="ExternalOutput").ap()

    with tile.TileContext(nc) as tc:
        with (
            tc.tile_pool(name="const", bufs=1) as const_pool,
            tc.tile_pool(name="state", bufs=1) as state_pool,
            tc.tile_pool(name="hT", bufs=4) as hT_pool,
            tc.tile_pool(name="hnat", bufs=4) as hn_pool,
            tc.tile_pool(name="psL", bufs=3, space="PSUM") as psL_pool,
            tc.tile_pool(name="psO", bufs=1, space="PSUM") as psO_pool,
            tc.tile_pool(name="psP", bufs=2, space="PSUM") as psP_pool,
            tc.tile_pool(name="ptile", bufs=2) as p_pool,
            tc.tile_pool(name="small", bufs=4) as small_pool,
        ):
            # ---- tile 0 DMAs (carrying qp / ident), then negM ----
            hT0 = const_pool.tile([128, QC + NCD * tiles[0]], F16, tag="hT0")
            nc.sync.dma_start(out=hT0[:], in_=hTp[:, : QC + NCD * tiles[0]])
            hn0 = const_pool.tile(
                [128, IC + (tiles[0] // 128) * DM], BF, tag="hn0"
            )
            nc.sync.dma_start(
                out=hn0[:], in_=hnp[:, : IC + (tiles[0] // 128) * DM]
            )
            negM_sb = const_pool.tile([K, 1], FP, tag="negM")
            nc.sync.dma_start(out=negM_sb[:], in_=negM)

            qp_sb = hT0  # columns 0:QC
            ident_sb = hn0  # rows 0:K, columns 0:IC

            denom = state_pool.tile([K, 1], FP, tag="denom")
            nc.vector.memset(denom[:], 0.0)
            # weighted sum accumulates into one persistent PSUM tile per
            # 512-wide d-half (separate tiles so finalizing one half does
            # not serialize against the other half's accumulation)
            o_ps = [
                psO_pool.tile([K, 512], FP, tag=f"psO{dh}", name=f"o_ps{dh}")
                for dh in range(NDH)
            ]
            out_sb = state_pool.tile([K, DM], FP, tag="out_sb")

            hT_off = [QC]
            hn_off = [IC]
            for st in tiles:
                hT_off.append(hT_off[-1] + NCD * st)
                hn_off.append(hn_off[-1] + (st // 128) * DM)

            def issue_dma(t):
                st = tiles[t]
                hT = hT_pool.tile([128, NCD * ST], F16, tag="hT")
                nc.sync.dma_start(
                    out=hT[:, : NCD * st],
                    in_=hTp[:, hT_off[t] : hT_off[t + 1]],
                )
                hn = hn_pool.tile([128, SUB * DM], BF, tag="hn")
                nc.sync.dma_start(
                    out=hn[:, : (st // 128) * DM],
                    in_=hnp[:, hn_off[t] : hn_off[t + 1]],
                )
                return hT, hn

            def mm1(t, hT, base):
                st = tiles[t]
                L = psL_pool.tile([K, ST], FP, tag="psL")
                for j in range(NCD):
                    nc.tensor.matmul(
                        L[:, :st],
                        qp_sb[:, j * K : (j + 1) * K],
                        hT[:, base + j * st : base + (j + 1) * st],
                        start=(j == 0),
                        stop=(j == NCD - 1),
                    )
                return L

            def finalize_half(dh):
                rden = small_pool.tile([K, 1], FP, tag=f"rden{dh}")
                nc.vector.reciprocal(rden[:], denom[:])
                nc.scalar.activation(
                    out_sb[:, dh * 512 : (dh + 1) * 512],
                    o_ps[dh][:],
                    mybir.ActivationFunctionType.Copy,
                    scale=rden[:],
                )
                if dh == NDH - 1:
                    nc.sync.dma_start(out=out, in_=out_sb[:])

            def tail(t, L, hn, base):
                st = tiles[t]
                sub = st // 128
                last = t == n_tiles - 1
                # p = exp(L - M) rounded straight to bf16; fp32 row sums
                # feed the denom (M is a host-computed per-row upper bound)
                p2 = p_pool.tile([K, ST], BF, tag="p2")
                tsum = small_pool.tile([K, 1], FP, tag="tsum")
                nc.scalar.activation(
                    p2[:, :st],
                    L[:, :st],
                    mybir.ActivationFunctionType.Exp,
                    bias=negM_sb[:],
                    accum_out=tsum[:],
                )
                nc.vector.tensor_add(denom[:], denom[:], tsum[:])

                # transpose p (all sub-chunks into one PSUM tile, one copy)
                tpp = psP_pool.tile([128, SUB * K], BF, tag="psP")
                for c in range(sub):
                    nc.tensor.transpose(
                        tpp[:, c * K : (c + 1) * K],
                        p2[:, c * 128 : (c + 1) * 128],
                        ident_sb[0:K, 0:K],
                    )
                pT = p_pool.tile([128, SUB * K], BF, tag="pT")
                nc.scalar.copy(pT[:, : sub * K], tpp[:, : sub * K])

                # weighted sum into the persistent PSUM accumulation groups;
                # on the last tile each dh group stops and finalizes eagerly
                for dh in range(NDH):
                    for c in range(sub):
                        nc.tensor.matmul(
                            o_ps[dh][:],
                            pT[:, c * K : (c + 1) * K],
                            hn[
                                :,
                                base + c * DM + dh * 512 : base
                                + c * DM
                                + dh * 512
                                + 512,
                            ],
                            start=(t == 0 and c == 0),
                            stop=(last and c == sub - 1),
                        )
                    if last:
                        finalize_half(dh)

            # mm1 runs two tiles ahead of the exp/transpose/weighted-sum tail
            # so the in-order PE reaches the final tile's logits (and the Act
            # chain behind them) as soon as its data lands, instead of after
            # the previous tile's weighted sum.
            depth = min(2, n_tiles - 1)
            pend = {0: (mm1(0, hT0, QC), hn0, IC)}
            for t in range(1, depth + 1):
                hT, hn = issue_dma(t)
                pend[t] = (mm1(t, hT, 0), hn, 0)
            for t in range(depth + 1, n_tiles):
                hT, hn = issue_dma(t)
                L = mm1(t, hT, 0)
                tail(t - depth - 1, *pend.pop(t - depth - 1))
                pend[t] = (L, hn, 0)
            for t in sorted(pend):
                tail(t, *pend.pop(t))

    nc.compile()
    return nc


_CACHED = {}


def _get_program(tiles):
    if tiles not in _CACHED:
        _CACHED[tiles] = build_program(tiles)
    return _CACHED[tiles]


def _tiles_for(mask):
    """Tile plan: full 512-row tiles plus a trailing 128-multiple tile."""
    n_max = int(np.asarray(mask).sum(axis=1).max())
    n_max = max(n_max, 128)
    nfull, rem = divmod(n_max, ST)
    tiles = (ST,) * nfull
    if rem:
        tiles = tiles + (math.ceil(rem / 128) * 128,)
    return tiles


def make_in_maps(hidden, mask, query, tiles):
    """Host staging: compact unmasked rows, pad to sum(tiles), both layouts."""
    hidden = np.ascontiguousarray(hidden, dtype=np.float32)
    mask = np.asarray(mask)
    query = np.asarray(query, dtype=np.float32)
    b, s, dm = hidden.shape
    k = query.shape[0]
    s_pad = sum(tiles)

    q16 = query.astype(np.float16)
    qp = np.ascontiguousarray(
        q16.T.reshape(NCD, 128, k).transpose(1, 0, 2).reshape(128, NCD * k)
    )
    ident = np.zeros((128, k), dtype=BF_NP)
    ident[:k, :k] = np.eye(k, dtype=BF_NP)

    rngM = np.random.default_rng(12345)
    in_maps = []
    for i in range(b):
        idx = np.flatnonzero(mask[i])
        n = len(idx)
        h = hidden[i][idx]                                 # [n, DM] f32

        # Per-row exp-shift bound M from a 512-row logit sample (+30
        # margin); stays far inside fp32/bf16 exp range either way.
        sidx = rngM.choice(n, min(512, n), replace=False)
        ls = query @ h[sidx].T                             # [K, <=512]
        M = np.maximum(ls.max(axis=1) + 30.0, 60.0)

        h16 = np.zeros((s_pad, dm), np.float16)
        h16[:n] = h
        hb = np.zeros((s_pad, dm), BF_NP)
        hb[:n] = h.astype(BF_NP)
        hT = h16.T                                         # [DM, s_pad]
        # per tile: hTp block [128, NCD*st] with col (j*st + si) holding
        # hT[j*128 + p, s0 + si]; hnp block [128, sub*DM] with col
        # (c*DM + d) holding hb[s0 + c*128 + p, d]
        hT_blocks, hn_blocks = [], []
        s0 = 0
        for st in tiles:
            sub = st // 128
            hT_blocks.append(
                hT[:, s0 : s0 + st]
                .reshape(NCD, 128, st)
                .transpose(1, 0, 2)
                .reshape(128, NCD * st)
            )
            hn_blocks.append(
                hb[s0 : s0 + st]
                .reshape(sub, 128, dm)
                .transpose(1, 0, 2)
                .reshape(128, sub * dm)
            )
            s0 += st
        in_maps.append(
            {
                "hTp": np.ascontiguousarray(
                    np.concatenate([qp] + hT_blocks, axis=1)
                ),
                "hnp": np.ascontiguousarray(
                    np.concatenate([ident] + hn_blocks, axis=1)
                ),
                "negM": (-M).astype(np.float32).reshape(k, 1),
            }
        )
    return in_maps


class _Runner:
    """jit-once SPMD runner (mirrors bass2jax.run_bass_via_pjrt, but reusable
    across calls so repeated invocations don't re-trace/re-compile)."""

    def __init__(self, nc):
        import jax
        from jax.sharding import Mesh, PartitionSpec, NamedSharding
        from jax.experimental.shard_map import shard_map
        from concourse.bass2jax import (
            _bass_exec_p,
            install_neuronx_cc_hook,
            partition_id_tensor,
        )

        install_neuronx_cc_hook()
        self.jax = jax
        partition_name = (
            nc.partition_id_tensor.name if nc.partition_id_tensor else None
        )
        in_names, out_names, out_avals, zero_outs = [], [], [], []
        for alloc in nc.m.functions[0].allocations:
            if not isinstance(alloc, mybir.MemoryLocationSet):
                continue
            name = alloc.memorylocations[0].name
            if alloc.kind == "ExternalInput":
                if name != partition_name:
                    in_names.append(name)
            elif alloc.kind == "ExternalOutput":
                out_names.append(name)
                shape = tuple(alloc.tensor_shape)
                dtype = mybir.dt.np(alloc.dtype)
                out_avals.append(jax.core.ShapedArray(shape, dtype))
                zero_outs.append(np.zeros(shape, dtype))
        self.in_names, self.out_names = in_names, out_names
        self.out_avals, self.zero_outs = out_avals, zero_outs
        n_params, n_outs = len(in_names), len(out_names)
        all_in_names = in_names + out_names
        if partition_name is not None:
            all_in_names = all_in_names + [partition_name]
        all_in_names = tuple(all_in_names)

        def _body(*args):
            operands = list(args)
            if partition_name is not None:
                operands.append(partition_id_tensor())
            outs = _bass_exec_p.bind(
                *operands,
                out_avals=tuple(out_avals),
                in_names=all_in_names,
                out_names=tuple(out_names),
                lowering_input_output_aliases=(),
                sim_require_finite=True,
                sim_require_nnan=True,
                nc=nc,
            )
            return tuple(outs)

        devices = jax.devices()[:N_CORES]
        self.mesh = Mesh(np.asarray(devices), ("core",))
        in_specs = (PartitionSpec("core"),) * (n_params + n_outs)
        out_specs = (PartitionSpec("core"),) * n_outs
        self.fn = jax.jit(
            shard_map(
                _body,
                mesh=self.mesh,
                in_specs=in_specs,
                out_specs=out_specs,
                check_rep=False,
            ),
            donate_argnums=tuple(range(n_params, n_params + n_outs)),
            keep_unused=True,
        )
        self.sharding = NamedSharding(self.mesh, PartitionSpec("core"))
        self._dev_in = None
        self._dev_in_key = None

    def put_inputs(self, in_maps):
        key = id(in_maps)
        if self._dev_in_key == key:
            return self._dev_in
        concat_in = [
            np.concatenate([m[name] for m in in_maps], axis=0)
            for name in self.in_names
        ]
        self._dev_in = [self.jax.device_put(x, self.sharding) for x in concat_in]
        self._dev_in_key = key
        return self._dev_in

    def run(self, in_maps):
        dev_in = self.put_inputs(in_maps)
        dev_zero = [
            self.jax.device_put(
                np.zeros((N_CORES * z.shape[0], *z.shape[1:]), z.dtype),
                self.sharding,
            )
            for z in self.zero_outs
        ]
        outs = self.fn(*dev_in, *dev_zero)
        self.jax.block_until_ready(outs)
        return {
            name: np.asarray(outs[i]).reshape(
                N_CORES, *self.out_avals[i].shape
            )
            for i, name in enumerate(self.out_names)
        }


_RUNNERS = {}


def _get_runner(tiles):
    if tiles not in _RUNNERS:
        _RUNNERS[tiles] = _Runner(_get_program(tiles))
    return _RUNNERS[tiles]


def kernel(hidden, mask, query):
    tiles = _tiles_for(mask)
    runner = _get_runner(tiles)
    in_maps = make_in_maps(hidden, mask, query, tiles)
    out = runner.run(in_maps)["out"]
    return out.astype(np.float32)


# revision 28
# speedup vs baseline: 3.8240x; 1.0652x over previous
"""AttentionPooling Trainium2 kernel.

Problem (per full input):
    hidden [B=8, S=8192, DM=1024] f32, mask [B, S] bool, query [K=8, DM] f32
    logits = einsum('kd,bsd->bks', query, hidden); masked (-1e4) softmax over S
    out    = einsum('bks,bsd->bkd', attn, hidden)              -> [B, K, DM] f32

Sharding: data-parallel over batch B; core i handles batch i. No collectives.

Key optimizations vs the bf16 hi/lo baseline:
  1. Host compaction: masked rows contribute exactly 0 to the softmax (the
     reference's -1e4 penalty underflows exp to 0.0 in fp32), so only the
     unmasked rows (~50%) are shipped, zero-padded to a whole number of
     512-row tiles. Padding rows have h == 0 and logit 0, so exp(0 - M)
     (M >= 60) contributes ~1e-27 to the denom and exactly 0 to the output.
  2. Single-pass fp16 logits matmul (fp16 = 1 cyc/row on PE, 11 mantissa
     bits) instead of a 2-pass bf16 hi/lo split; weighted-sum matmul in
     bf16 with the attention weights split hi/lo (bf16 range is needed
     because unnormalized p can reach ~e^30). Validated end-to-end error
     ~7e-3 vs the 2e-2 gate.
  Net per-core HBM traffic: 64 MB -> ~19 MB; PE rows: 262k -> ~74k.

Host staging ships both layouts of the compacted rows ([D,S] fp16 for the
logits matmul, [S,D] bf16 for the weighted sum), pre-swizzled so each
512-row tile is one 8 KB/partition contiguous DMA. The exp shift M is a
host-computed per-row upper bound (512-row sampled logits + 30 margin), so
no on-chip running max / rescale chain is needed.
"""

import math
import sys

import numpy as np

sys.path.insert(0, "/opt/trn_rl_repo")

import ml_dtypes

import concourse.tile as tile
from concourse import bacc, mybir

FP = mybir.dt.float32
BF = mybir.dt.bfloat16
F16 = mybir.dt.float16
BF_NP = ml_dtypes.bfloat16

# Problem config (hardcoded; harness calls kernel() with exactly these shapes)
B, S, DM, K = 8, 8192, 1024, 8
N_CORES = 8
ST = 512                   # s-tile rows (one PSUM bank for the logits tile)
SUB = ST // 128            # 128-row subchunks per s-tile
NCD = DM // 128            # 128-d chunks for the logits matmul
NDH = DM // 512            # 512-wide d halves for the weighted-sum matmul


def build_program(tiles):
    """Build the per-core Bass program.

    tiles: tuple of s-tile row counts (multiples of 128, at most ST each),
    e.g. (512,)*8 + (256,) for 4352 compacted rows.

    The loop is software-pipelined: tile t's logits matmul is issued on the
    PE before tile t-1's exp -> transpose -> weighted-sum chain, so the
    in-order PE never stalls waiting on the Act chain. The tiny constants
    (q stationary / transpose identity) ride along inside tile 0's two big
    DMAs so no extra DMA issue delays the stream head.
    """
    n_tiles = len(tiles)
    transposed = _transposed_set(tiles)
    QC = NCD * K + 128    # qp + 128x128 fp16 identity before tile 0's hT block
    IC = K                # ident columns prepended to tile 0's hn block
    hT_cols = QC + sum(NCD * st for st in tiles)
    hn_cols = IC + sum(
        (st // 128) * DM for i, st in enumerate(tiles) if i not in transposed
    )

    nc = bacc.Bacc(
        "TRN2",
        target_bir_lowering=False,
        debug=False,
        num_devices=N_CORES,
    )

    hTp = nc.dram_tensor("hTp", [128, hT_cols], F16, kind="ExternalInput").ap()
    hnp = nc.dram_tensor("hnp", [128, hn_cols], BF, kind="ExternalInput").ap()
    negM = nc.dram_tensor("negM", [K, 1], FP, kind="ExternalInput").ap()
    out = nc.dram_tensor("out", [K, DM], FP, kind="ExternalOutput").ap()

    with tile.TileContext(nc) as tc:
        with (
            tc.tile_pool(name="const", bufs=1) as const_pool,
            tc.tile_pool(name="state", bufs=1) as state_pool,
            tc.tile_pool(name="hT", bufs=4) as hT_pool,
            tc.tile_pool(name="hnat", bufs=4) as hn_pool,
            tc.tile_pool(name="psL", bufs=3, space="PSUM") as psL_pool,
            tc.tile_pool(name="psO", bufs=1, space="PSUM") as psO_pool,
            tc.tile_pool(name="psP", bufs=1, space="PSUM") as psP_pool,
            tc.tile_pool(name="psT", bufs=2, space="PSUM") as psT_pool,
            tc.tile_pool(name="ptile", bufs=2) as p_pool,
            tc.tile_pool(name="small", bufs=4) as small_pool,
        ):
            # ---- tile 0 DMAs (carrying qp / ident), then negM ----
            hT0 = const_pool.tile([128, QC + NCD * tiles[0]], F16, tag="hT0")
            nc.sync.dma_start(out=hT0[:], in_=hTp[:, : QC + NCD * tiles[0]])
            hn0 = const_pool.tile(
                [128, IC + (tiles[0] // 128) * DM], BF, tag="hn0"
            )
            nc.sync.dma_start(
                out=hn0[:], in_=hnp[:, : IC + (tiles[0] // 128) * DM]
            )
            negM_sb = const_pool.tile([K, 1], FP, tag="negM")
            nc.sync.dma_start(out=negM_sb[:], in_=negM)

            qp_sb = hT0  # columns 0:NCD*K, then the 128x128 fp16 identity
            ident_sb = hn0  # rows 0:K, columns 0:IC

            denom = state_pool.tile([K, 1], FP, tag="denom")
            nc.vector.memset(denom[:], 0.0)
            # weighted sum accumulates into one persistent PSUM tile per
            # 512-wide d-half (separate tiles so finalizing one half does
            # not serialize against the other half's accumulation)
            o_ps = [
                psO_pool.tile([K, 512], FP, tag=f"psO{dh}", name=f"o_ps{dh}")
                for dh in range(NDH)
            ]
            out_sb = state_pool.tile([K, DM], FP, tag="out_sb")

            hT_off = [QC]
            hn_off = [IC]
            for i, st in enumerate(tiles):
                hT_off.append(hT_off[-1] + NCD * st)
                hn_off.append(
                    hn_off[-1]
                    + ((st // 128) * DM if i not in transposed else 0)
                )

            def issue_dma(t):
                st = tiles[t]
                hT = hT_pool.tile([128, NCD * ST], F16, tag="hT")
                nc.sync.dma_start(
                    out=hT[:, : NCD * st],
                    in_=hTp[:, hT_off[t] : hT_off[t + 1]],
                )
                hn = hn_pool.tile([128, SUB * DM], BF, tag="hn")
                if t not in transposed:
                    nc.sync.dma_start(
                        out=hn[:, : (st // 128) * DM],
                        in_=hnp[:, hn_off[t] : hn_off[t + 1]],
                    )
                return hT, hn

            def build_hn_on_chip(t, hT, hn):
                # hn[x, c*DM + j*128 + p] = h[c*128+x, j*128+p]
                #   = transpose of hT[:, j*st + c*128 : j*st + (c+1)*128];
                # 4 j-blocks per PSUM tile -> one 512-wide converting copy,
                # alternating Act/DVE so neither engine becomes the bottleneck
                st = tiles[t]
                for c in range(st // 128):
                    for half in range(2):
                        psT = psT_pool.tile([128, 512], F16, tag="psT")
                        for jj in range(4):
                            j = half * 4 + jj
                            nc.tensor.transpose(
                                psT[:, jj * 128 : (jj + 1) * 128],
                                hT[:, j * st + c * 128 : j * st + (c + 1) * 128],
                                qp_sb[:, NCD * K : NCD * K + 128],
                            )
                        dst = hn[:, c * DM + half * 512 : c * DM + half * 512 + 512]
                        if half == 0:
                            nc.scalar.copy(dst, psT[:])
                        else:
                            nc.vector.tensor_copy(dst, psT[:])

            def mm1(t, hT, base):
                st = tiles[t]
                L = psL_pool.tile([K, ST], FP, tag="psL")
                for j in range(NCD):
                    nc.tensor.matmul(
                        L[:, :st],
                        qp_sb[:, j * K : (j + 1) * K],
                        hT[:, base + j * st : base + (j + 1) * st],
                        start=(j == 0),
                        stop=(j == NCD - 1),
                    )
                return L

            def finalize_half(dh):
                rden = small_pool.tile([K, 1], FP, tag=f"rden{dh}")
                nc.vector.reciprocal(rden[:], denom[:])
                nc.scalar.activation(
                    out_sb[:, dh * 512 : (dh + 1) * 512],
                    o_ps[dh][:],
                    mybir.ActivationFunctionType.Copy,
                    scale=rden[:],
                )
                if dh == NDH - 1:
                    nc.sync.dma_start(out=out, in_=out_sb[:])

            def tail(t, L, hn, base):
                st = tiles[t]
                sub = st // 128
                last = t == n_tiles - 1
                # p = exp(L - M) rounded straight to bf16; fp32 row sums
                # feed the denom (M is a host-computed per-row upper bound)
                p2 = p_pool.tile([K, ST], BF, tag="p2")
                tsum = small_pool.tile([K, 1], FP, tag="tsum")
                nc.scalar.activation(
                    p2[:, :st],
                    L[:, :st],
                    mybir.ActivationFunctionType.Exp,
                    bias=negM_sb[:],
                    accum_out=tsum[:],
                )
                nc.vector.tensor_add(denom[:], denom[:], tsum[:])

                # transpose p (all sub-chunks into one PSUM tile, one copy)
                tpp = psP_pool.tile([128, SUB * K], BF, tag="psP")
                for c in range(sub):
                    nc.tensor.transpose(
                        tpp[:, c * K : (c + 1) * K],
                        p2[:, c * 128 : (c + 1) * 128],
                        ident_sb[0:K, 0:K],
                    )
                pT = p_pool.tile([128, SUB * K], BF, tag="pT")
                nc.scalar.copy(pT[:, : sub * K], tpp[:, : sub * K])

                # weighted sum into the persistent PSUM accumulation groups;
                # on the last tile each dh group stops and finalizes eagerly
                for dh in range(NDH):
                    for c in range(sub):
                        nc.tensor.matmul(
                            o_ps[dh][:],
                            pT[:, c * K : (c + 1) * K],
                            hn[
                                :,
                                base + c * DM + dh * 512 : base
                                + c * DM
                                + dh * 512
                                + 512,
                            ],
                            start=(t == 0 and c == 0),
                            stop=(last and c == sub - 1),
                        )
                    if last:
                        finalize_half(dh)

            # mm1 runs two tiles ahead of the exp/transpose/weighted-sum tail
            # so the in-order PE reaches the final tile's logits (and the Act
            # chain behind them) as soon as its data lands, instead of after
            # the previous tile's weighted sum.
            depth = min(2, n_tiles - 1)
            pend = {0: (mm1(0, hT0, QC), hn0, IC)}
            for t in range(1, depth + 1):
                hT, hn = issue_dma(t)
                pend[t] = (mm1(t, hT, 0), hn, 0)
                if t in transposed:
                    build_hn_on_chip(t, hT, hn)
            for t in range(depth + 1, n_tiles):
                hT, hn = issue_dma(t)
                L = mm1(t, hT, 0)
                if t in transposed:
                    build_hn_on_chip(t, hT, hn)
                tail(t - depth - 1, *pend.pop(t - depth - 1))
                pend[t] = (L, hn, 0)
            for t in sorted(pend):
                tail(t, *pend.pop(t))

    nc.compile()
    return nc


_CACHED = {}


def _get_program(tiles):
    if tiles not in _CACHED:
        _CACHED[tiles] = build_program(tiles)
    return _CACHED[tiles]


def _tiles_for(mask):
    """Tile plan: full 512-row tiles plus a trailing 128-multiple tile."""
    n_max = int(np.asarray(mask).sum(axis=1).max())
    n_max = max(n_max, 128)
    nfull, rem = divmod(n_max, ST)
    tiles = (ST,) * nfull
    if rem:
        tiles = tiles + (math.ceil(rem / 128) * 128,)
    return tiles


def _transposed_set(tiles):
    """Mid-stream full tiles whose [S,D]-layout operand is built on-chip by
    PE-transposing the already-shipped [D,S] fp16 data (saves its hn DMA).
    Tiles 0-1 (pipeline warmup) and the last two (latency tail) still ship."""
    return frozenset(
        i
        for i in (2, 4, 6)
        if i < len(tiles) - 2 and tiles[i] == ST
    )


def make_in_maps(hidden, mask, query, tiles):
    """Host staging: compact unmasked rows, pad to sum(tiles), both layouts."""
    hidden = np.ascontiguousarray(hidden, dtype=np.float32)
    mask = np.asarray(mask)
    query = np.asarray(query, dtype=np.float32)
    b, s, dm = hidden.shape
    k = query.shape[0]
    s_pad = sum(tiles)

    transposed = _transposed_set(tiles)
    q16 = query.astype(np.float16)
    qp = np.concatenate(
        [
            q16.T.reshape(NCD, 128, k).transpose(1, 0, 2).reshape(128, NCD * k),
            np.eye(128, dtype=np.float16),
        ],
        axis=1,
    )
    ident = np.zeros((128, k), dtype=BF_NP)
    ident[:k, :k] = np.eye(k, dtype=BF_NP)

    rngM = np.random.default_rng(12345)
    in_maps = []
    for i in range(b):
        idx = np.flatnonzero(mask[i])
        n = len(idx)
        h = hidden[i][idx]                                 # [n, DM] f32

        # Per-row exp-shift bound M from a 512-row logit sample (+30
        # margin); stays far inside fp32/bf16 exp range either way.
        sidx = rngM.choice(n, min(512, n), replace=False)
        ls = query @ h[sidx].T                             # [K, <=512]
        M = np.maximum(ls.max(axis=1) + 30.0, 60.0)

        h16 = np.zeros((s_pad, dm), np.float16)
        h16[:n] = h
        hb = np.zeros((s_pad, dm), BF_NP)
        hb[:n] = h.astype(BF_NP)
        hT = h16.T                                         # [DM, s_pad]
        # per tile: hTp block [128, NCD*st] with col (j*st + si) holding
        # hT[j*128 + p, s0 + si]; hnp block [128, sub*DM] with col
        # (c*DM + d) holding hb[s0 + c*128 + p, d]
        hT_blocks, hn_blocks = [], []
        s0 = 0
        for ti, st in enumerate(tiles):
            sub = st // 128
            hT_blocks.append(
                hT[:, s0 : s0 + st]
                .reshape(NCD, 128, st)
                .transpose(1, 0, 2)
                .reshape(128, NCD * st)
            )
            if ti not in transposed:
                hn_blocks.append(
                    hb[s0 : s0 + st]
                    .reshape(sub, 128, dm)
                    .transpose(1, 0, 2)
                    .reshape(128, sub * dm)
                )
            s0 += st
        in_maps.append(
            {
                "hTp": np.ascontiguousarray(
                    np.concatenate([qp] + hT_blocks, axis=1)
                ),
                "hnp": np.ascontiguousarray(
                    np.concatenate([ident] + hn_blocks, axis=1)
                ),
                "negM": (-M).astype(np.float32).reshape(k, 1),
            }
        )
    return in_maps


class _Runner:
    """jit-once SPMD runner (mirrors bass2jax.run_bass_via_pjrt, but reusable
    across calls so repeated invocations don't re-trace/re-compile)."""

    def __init__(self, nc):
        import jax
        from jax.sharding import Mesh, PartitionSpec, NamedSharding
        from jax.experimental.shard_map import shard_map
        from concourse.bass2jax import (
            _bass_exec_p,
            install_neuronx_cc_hook,
            partition_id_tensor,
        )

        install_neuronx_cc_hook()
        self.jax = jax
        partition_name = (
            nc.partition_id_tensor.name if nc.partition_id_tensor else None
        )
        in_names, out_names, out_avals, zero_outs = [], [], [], []
        for alloc in nc.m.functions[0].allocations:
            if not isinstance(alloc, mybir.MemoryLocationSet):
                continue
            name = alloc.memorylocations[0].name
            if alloc.kind == "ExternalInput":
                if name != partition_name:
                    in_names.append(name)
            elif alloc.kind == "ExternalOutput":
                out_names.append(name)
                shape = tuple(alloc.tensor_shape)
                dtype = mybir.dt.np(alloc.dtype)
                out_avals.append(jax.core.ShapedArray(shape, dtype))
                zero_outs.append(np.zeros(shape, dtype))
        self.in_names, self.out_names = in_names, out_names
        self.out_avals, self.zero_outs = out_avals, zero_outs
        n_params, n_outs = len(in_names), len(out_names)
        all_in_names = in_names + out_names
        if partition_name is not None:
            all_in_names = all_in_names + [partition_name]
        all_in_names = tuple(all_in_names)

        def _body(*args):
            operands = list(args)
            if partition_name is not None:
                operands.append(partition_id_tensor())
            outs = _bass_exec_p.bind(
                *operands,
                out_avals=tuple(out_avals),
                in_names=all_in_names,
                out_names=tuple(out_names),
                lowering_input_output_aliases=(),
                sim_require_finite=True,
                sim_require_nnan=True,
                nc=nc,
            )
            return tuple(outs)

        devices = jax.devices()[:N_CORES]
        self.mesh = Mesh(np.asarray(devices), ("core",))
        in_specs = (PartitionSpec("core"),) * (n_params + n_outs)
        out_specs = (PartitionSpec("core"),) * n_outs
        self.fn = jax.jit(
            shard_map(
                _body,
                mesh=self.mesh,
                in_specs=in_specs,
                out_specs=out_specs,
                check_rep=False,
            ),
            donate_argnums=tuple(range(n_params, n_params + n_outs)),
            keep_unused=True,
        )
        self.sharding = NamedSharding(self.mesh, PartitionSpec("core"))
        self._dev_in = None
        self._dev_in_key = None

    def put_inputs(self, in_maps):
        key = id(in_maps)
        if self._dev_in_key == key:
            return self._dev_in
        concat_in = [
            np.concatenate([m[name] for m in in_maps], axis=0)
            for name in self.in_names
        ]
        self._dev_in = [self.jax.device_put(x, self.sharding) for x in concat_in]
        self._dev_in_key = key
        return self._dev_in

    def run(self, in_maps):
        dev_in = self.put_inputs(in_maps)
        dev_zero = [
            self.jax.device_put(
                np.zeros((N_CORES * z.shape[0], *z.shape[1:]), z.dtype),
                self.sharding,
            )
            for z in self.zero_outs
        ]
        outs = self.fn(*dev_in, *dev_zero)
        self.jax.block_until_ready(outs)
        return {
            name: np.asarray(outs[i]).reshape(
                N_CORES, *self.out_avals[i].shape
            )
            for i, name in enumerate(self.out_names)
        }


_RUNNERS = {}


def _get_runner(tiles):
    if tiles not in _RUNNERS:
        _RUNNERS[tiles] = _Runner(_get_program(tiles))
    return _RUNNERS[tiles]


def kernel(hidden, mask, query):
    tiles = _tiles_for(mask)
    runner = _get_runner(tiles)
    in_maps = make_in_maps(hidden, mask, query, tiles)
    out = runner.run(in_maps)["out"]
    return out.astype(np.float32)
